# revision 1
# baseline (speedup 1.0000x reference)
"""Bass/Tile TRN2 kernel for nn_Attention_26388279067013.

Computes, for each batch row b:
    feat = enc @ We.T + dec @ Ws.T + cov[:,None] * Wc.sum(1) + b     [S, H]
    att  = tanh(feat) @ v_w                                          [S]
    att[s >= L_b] = -inf ; w = softmax(att) ; new_cov = cov + w
Returns (attention_weights [B,S], new_coverage [B,S]) both float32.

Sharding: data-parallel over B across 8 NeuronCores (4 rows each).
"""

import sys

sys.path.insert(0, "/opt/trn_rl_repo")

import numpy as np

import concourse.bacc as bacc
import concourse.tile as tile
import concourse.mybir as mybir
from concourse.bass_utils import run_bass_kernel_spmd

B, S, H, D = 32, 4096, 512, 256
N_CORES = 8
B_LOC = B // N_CORES          # 4 batch rows per core
F32 = mybir.dt.float32
F32R = mybir.dt.float32r
ALU = mybir.AluOpType
ACTF = mybir.ActivationFunctionType

N_K = H // 128                # 4 contraction tiles
N_STILE = S // 128            # 32 psum s-tiles per batch
N_CHUNK = S // 512            # 8 dma chunks per batch
NEG_BIG = -30000.0            # exp(x - 30000) == 0.0 exactly in f32


def r32(ap):
    return ap.bitcast(F32R)


def build_kernel():
    nc = bacc.Bacc("TRN2", debug=False, num_devices=N_CORES)

    # ---- dram I/O (per core) ----
    encT = nc.dram_tensor("encT", [B_LOC, H, S], F32, kind="ExternalInput").ap()
    cov = nc.dram_tensor("cov", [B_LOC, 32, 128], F32, kind="ExternalInput").ap()
    dec_cols = nc.dram_tensor("dec_cols", [B_LOC, 128, 2], F32, kind="ExternalInput").ap()
    lens = nc.dram_tensor("lens", [B_LOC, 1], F32, kind="ExternalInput").ap()
    WeT = nc.dram_tensor("WeT", [H, H], F32, kind="ExternalInput").ap()
    WcT = nc.dram_tensor("WcT", [H, H], F32, kind="ExternalInput").ap()
    WsT = nc.dram_tensor("WsT", [D, H], F32, kind="ExternalInput").ap()
    b_row = nc.dram_tensor("b_row", [1, H], F32, kind="ExternalInput").ap()
    v_row = nc.dram_tensor("v_row", [1, H], F32, kind="ExternalInput").ap()
    iota_d = nc.dram_tensor("iota_pm", [128, 32], F32, kind="ExternalInput").ap()
    ident_d = nc.dram_tensor("ident", [128, 128], F32, kind="ExternalInput").ap()
    ones_row = nc.dram_tensor("ones_row", [1, S], F32, kind="ExternalInput").ap()
    out_w = nc.dram_tensor("out_w", [B_LOC, 32, 128], F32, kind="ExternalOutput").ap()
    out_c = nc.dram_tensor("out_c", [B_LOC, 32, 128], F32, kind="ExternalOutput").ap()

    with tile.TileContext(nc) as tc:
        with (
            tc.tile_pool(name="persist", bufs=1) as pp,
            tc.tile_pool(name="enc", bufs=12) as encp,
            tc.tile_pool(name="x", bufs=3) as xp,
            tc.tile_pool(name="scratch", bufs=2) as scrp,
            tc.tile_pool(name="small", bufs=4) as smp,
            tc.tile_pool(name="batch", bufs=3) as bp,
            tc.tile_pool(name="psum", bufs=2, space="PSUM") as psp,
            tc.tile_pool(name="psum_misc", bufs=4, space="PSUM") as psm,
        ):
            # ---- one-time setup ----
            wet = []
            for k in range(N_K):
                t = pp.tile([128, H], F32R, tag=f"wet{k}")
                nc.scalar.dma_start(t[:], r32(WeT[k * 128:(k + 1) * 128, :]))
                wet.append(t)
            wst = []
            for k in range(D // 128):
                t = pp.tile([128, H], F32, tag=f"wst{k}")
                nc.scalar.dma_start(t[:], WsT[k * 128:(k + 1) * 128, :])
                wst.append(t)
            brow_sb = pp.tile([1, H], F32, tag="brow")
            nc.scalar.dma_start(brow_sb[:], b_row[:, :])
            vrow_sb = pp.tile([1, H], F32, tag="vrow")
            nc.scalar.dma_start(vrow_sb[:], v_row[:, :])
            ones_k1 = pp.tile([1, 128], F32, tag="ones_k1")
            nc.vector.memset(ones_k1[:], 1.0)
            ones_col = pp.tile([128, 1], F32, tag="ones_col")
            nc.vector.memset(ones_col[:], 1.0)

            # dep-free matmul burst: trips the PE HAM to K=8/8 (~2.4 GHz)
            # before the real stream arrives, instead of ~40us into it.
            warm_f = pp.tile([128, 512], F32, tag="warm_f")
            nc.vector.memset(warm_f[:], 0.5)
            warm = pp.tile([128, 512], F32R, tag="warm")
            nc.scalar.dma_start(warm[:], r32(warm_f[:]))
            for wi in range(20):
                ps_w = psm.tile([128, 512], F32, tag="mpsum")
                nc.tensor.matmul(ps_w[:], warm[:, 0:128], warm[:],
                                 start=True, stop=True)

            # wc_sum[o] = sum_h WcT[h, o]  -> [1, 512]
            ps_wc = psm.tile([1, H], F32, tag="mpsum")
            for k in range(N_K):
                t = scrp.tile([128, H], F32, tag="wct")
                nc.scalar.dma_start(t[:], WcT[k * 128:(k + 1) * 128, :])
                nc.tensor.matmul(ps_wc[:], ones_col[:], t[:],
                                 start=(k == 0), stop=(k == N_K - 1))
            wc_row = pp.tile([1, H], F32, tag="wc_row")
            nc.scalar.copy(wc_row[:], ps_wc[:])

            # v_bcast[p, o] = v_w[o]
            ps_vb = psm.tile([128, H], F32, tag="mpsum")
            nc.tensor.matmul(ps_vb[:], ones_k1[:], vrow_sb[:],
                             start=True, stop=True)
            v_bcast = pp.tile([128, H], F32, tag="v_bcast")
            nc.scalar.copy(v_bcast[:], ps_vb[:])

            iota_sb = pp.tile([128, 32], F32, tag="iota")
            ident_sb = pp.tile([128, 128], F32, tag="ident")

            # ---- per batch, software-pipelined ----
            # prep(b) builds per-batch small operands; heavy(b) is the matmul
            # stream; softmax(b) is emitted in the middle of heavy(b+1) so the
            # PE never drains at a batch boundary (keeps HAM warm).
            state = {}

            def emit_prep(b):
                dc = smp.tile([128, 2], F32, tag="dc")
                nc.scalar.dma_start(dc[:], dec_cols[b, :, :])
                ps_row = psm.tile([1, H], F32, tag="mpsum")
                for j in range(D // 128):
                    nc.tensor.matmul(ps_row[:], dc[:, j:j + 1], wst[j][:],
                                     start=(j == 0), stop=(j == 1))
                aug_st = bp.tile([2, H], F32, tag="aug_st")
                nc.vector.tensor_tensor(aug_st[0:1, :], ps_row[:], brow_sb[:], ALU.add)
                nc.scalar.dma_start(aug_st[1:2, :], wc_row[:])
                aug_rhs = bp.tile([2, H], F32R, tag="aug_rhs")
                nc.scalar.dma_start(aug_rhs[:], r32(aug_st[:]))

                cov_aug = bp.tile([2, S], F32R, tag="cov_aug")
                nc.scalar.dma_start(cov_aug[0:1, :], r32(ones_row[:, :]))
                nc.scalar.dma_start(
                    cov_aug[1:2, :],
                    r32(cov[b:b + 1].rearrange("c a b -> c (a b)")),
                )

                len_sb = smp.tile([1, 1], F32, tag="len_sb")
                nc.scalar.dma_start(len_sb[:], lens[b:b + 1, :])
                ps_l = psm.tile([128, 1], F32, tag="mpsum")
                nc.tensor.matmul(ps_l[:], ones_k1[:], len_sb[:],
                                 start=True, stop=True)
                l_col = smp.tile([128, 1], F32, tag="l_col")
                nc.scalar.copy(l_col[:], ps_l[:])

                att_pm = bp.tile([128, 32], F32, tag="att_pm")
                state[b] = dict(aug_rhs=aug_rhs, cov_aug=cov_aug,
                                l_col=l_col, att_pm=att_pm)

            def emit_heavy_chunk(b, c):
                st8 = state[b]
                ek = []
                for k in range(N_K):
                    t = encp.tile([128, 512], F32R, tag="enc")
                    nc.sync.dma_start(
                        t[:], r32(encT[b, k * 128:(k + 1) * 128, c * 512:(c + 1) * 512]))
                    ek.append(t)
                for t2 in range(2):
                    ps = psp.tile([128, 1024], F32, tag="feat")
                    for half in range(2):
                        st = 4 * c + 2 * t2 + half
                        scol = (2 * t2 + half) * 128
                        dst = ps[:, half * 512:(half + 1) * 512]
                        for k in range(N_K):
                            nc.tensor.matmul(
                                dst, ek[k][:, scol:scol + 128], wet[k][:],
                                start=(k == 0), stop=False)
                        nc.tensor.matmul(
                            dst, st8["cov_aug"][:, st * 128:(st + 1) * 128],
                            st8["aug_rhs"][:], start=False, stop=True)
                    x = xp.tile([128, 1024], F32, tag="x")
                    nc.scalar.activation(x[:], ps[:], ACTF.Tanh)
                    for half in range(2):
                        st = 4 * c + 2 * t2 + half
                        scr = scrp.tile([128, 512], F32, tag="vscr")
                        nc.vector.scalar_tensor_tensor(
                            scr[:], x[:, half * 512:(half + 1) * 512],
                            1.0, v_bcast[:], ALU.bypass, ALU.mult,
                            accum_out=st8["att_pm"][:, st:st + 1])

            def emit_softmax(b):
                st8 = state.pop(b)
                att_pm, l_col = st8["att_pm"], st8["l_col"]
                pad01 = bp.tile([128, 32], F32, tag="pad01")
                nc.vector.tensor_scalar(pad01[:], iota_sb[:], l_col[:], None, ALU.is_ge)
                att_m = bp.tile([128, 32], F32, tag="att_m")
                nc.vector.scalar_tensor_tensor(
                    att_m[:], pad01[:], NEG_BIG, att_pm[:], ALU.mult, ALU.add)
                exp_pm = bp.tile([128, 32], F32, tag="exp_pm")
                rowsum = smp.tile([128, 1], F32, tag="rowsum")
                nc.scalar.activation(exp_pm[:], att_m[:], ACTF.Exp, accum_out=rowsum[:])
                ps_d = psm.tile([1, 1], F32, tag="mpsum")
                nc.tensor.matmul(ps_d[:], rowsum[:], ones_col[:],
                                 start=True, stop=True)
                rinv = smp.tile([1, 1], F32, tag="rinv")
                nc.vector.reciprocal(rinv[:], ps_d[:])
                ps_r = psm.tile([128, 1], F32, tag="mpsum")
                nc.tensor.matmul(ps_r[:], ones_k1[:], rinv[:],
                                 start=True, stop=True)
                rinv_col = smp.tile([128, 1], F32, tag="rinv_col")
                nc.scalar.copy(rinv_col[:], ps_r[:])
                w_pm = bp.tile([128, 32], F32, tag="w_pm")
                nc.vector.tensor_scalar(w_pm[:], exp_pm[:], rinv_col[:], None, ALU.mult)

                ps_t = psm.tile([32, 128], F32, tag="mpsum")
                nc.tensor.transpose(ps_t[:], w_pm[:], ident_sb[:])
                covT = bp.tile([32, 128], F32, tag="covT")
                nc.scalar.dma_start(covT[:], cov[b, :, :])
                w_sb = bp.tile([32, 128], F32, tag="w_sb")
                nc.scalar.copy(w_sb[:], ps_t[:])
                ncov = bp.tile([32, 128], F32, tag="ncov")
                nc.vector.tensor_tensor(ncov[:], ps_t[:], covT[:], ALU.add)
                nc.scalar.dma_start(out_w[b, :, :], w_sb[:])
                nc.scalar.dma_start(out_c[b, :, :], ncov[:])

            emit_prep(0)
            emit_prep(1)
            nc.scalar.dma_start(iota_sb[:], iota_d[:, :])
            nc.scalar.dma_start(ident_sb[:], ident_d[:, :])
            for b in range(B_LOC):
                for c in range(N_CHUNK):
                    emit_heavy_chunk(b, c)
                    if c == 2 and b >= 1:
                        emit_softmax(b - 1)
                    if c == 5 and b + 2 < B_LOC:
                        emit_prep(b + 2)
            emit_softmax(B_LOC - 1)

    nc.compile()
    return nc


_NC_CACHE = {}


def _get_nc():
    if "nc" not in _NC_CACHE:
        _NC_CACHE["nc"] = build_kernel()
    return _NC_CACHE["nc"]


def make_in_maps(dec_input, enc_output, coverage_vector, text_lengths, W, b, v_w, v_b):
    dec_input = np.asarray(dec_input, np.float32)
    enc_output = np.ascontiguousarray(np.asarray(enc_output, np.float32))
    coverage_vector = np.asarray(coverage_vector, np.float32)
    lens_f = np.asarray(text_lengths).astype(np.float32)
    W = np.asarray(W, np.float32)
    b = np.asarray(b, np.float32)
    v_w = np.asarray(v_w, np.float32)

    WeT = np.ascontiguousarray(W[:, :H].T)            # [H, H]
    WsT = np.ascontiguousarray(W[:, H:H + D].T)       # [D, H]
    WcT = np.ascontiguousarray(W[:, H + D:].T)        # [H, H]
    b_rw = np.ascontiguousarray(b[None, :])
    v_rw = np.ascontiguousarray(v_w[None, :])
    iota_pm = (np.arange(32)[None, :] * 128 + np.arange(128)[:, None]).astype(np.float32)
    ident = np.eye(128, dtype=np.float32)

    in_maps = []
    for core in range(N_CORES):
        lo = core * B_LOC
        hi = lo + B_LOC
        encT = np.ascontiguousarray(enc_output[lo:hi].transpose(0, 2, 1))  # [B_LOC, H, S]
        covc = np.ascontiguousarray(coverage_vector[lo:hi].reshape(B_LOC, 32, 128))
        decc = np.ascontiguousarray(
            dec_input[lo:hi, 0, :].reshape(B_LOC, 2, 128).transpose(0, 2, 1))
        in_maps.append({
            "encT": encT,
            "cov": covc,
            "dec_cols": decc,
            "lens": np.ascontiguousarray(lens_f[lo:hi].reshape(B_LOC, 1)),
            "WeT": WeT, "WcT": WcT, "WsT": WsT,
            "b_row": b_rw, "v_row": v_rw,
            "iota_pm": iota_pm, "ident": ident,
            "ones_row": np.ones((1, S), np.float32),
        })
    return in_maps


def kernel(dec_input, enc_output, coverage_vector, text_lengths, W, b, v_w, v_b,
           _trace=False):
    nc = _get_nc()
    in_maps = make_in_maps(dec_input, enc_output, coverage_vector, text_lengths,
                           W, b, v_w, v_b)
    res = run_bass_kernel_spmd(nc, in_maps, list(range(N_CORES)), trace=_trace)
    w = np.concatenate([r["out_w"].reshape(B_LOC, S) for r in res.results], axis=0)
    c = np.concatenate([r["out_c"].reshape(B_LOC, S) for r in res.results], axis=0)
    if _trace:
        kernel.last_result = res
    return w, c



# revision 6
# speedup vs baseline: 1.5004x; 1.5004x over previous
"""Bass/Tile TRN2 kernel for nn_Attention_26388279067013 (v3).

Computes, for each batch row b:
    feat = enc @ We.T + dec @ Ws.T + cov[:,None] * Wc.sum(1) + b     [S, H]
    att  = tanh(feat) @ v_w                                          [S]
    att[s >= L_b] = -inf ; w = softmax(att) ; new_cov = cov + w

Key optimizations over the f32r baseline:
  - enc/We in fp8 e4m3 (x16 / x64 scaling) with DoubleRow matmuls:
    2 virtual-K=256 MMs per 128-row s-tile instead of 4 f32r MMs.
    The 1/1024 descale rides the tanh activation's free scale.
  - dec/bias/coverage rank-1 terms via one bf16 K=2 matmul per s-tile
    ([ones; cov] x [db; wc_sum], db = dec @ Ws.T + b computed on host).
  - masked positions (s >= L_b) have w == 0 exactly, so only
    ceil(L/128) s-tiles are computed. Batches are sorted by length and
    dealt round-robin to (core, slot) so the compiled per-slot tile
    counts (max over cores) stay small; host fills w=0 / c=cov for the
    skipped tail. new_cov = cov + w is a host-side add.
  - x = tanh(feat) in bf16 -> DVE scalar_tensor_tensor v-dot at 2x.

Sharding: 4 batch slots per core across 8 NeuronCores (SPMD).
"""

import sys

sys.path.insert(0, "/opt/trn_rl_repo")

import numpy as np
import ml_dtypes

import concourse.bacc as bacc
import concourse.tile as tile
import concourse.mybir as mybir
from concourse.bass_utils import run_bass_kernel_spmd

B, S, H, D = 32, 4096, 512, 256
N_CORES = 8
N_SLOTS = 4
F32 = mybir.dt.float32
BF16 = mybir.dt.bfloat16
F8 = mybir.dt.float8e4
ALU = mybir.AluOpType
ACTF = mybir.ActivationFunctionType
DR = mybir.MatmulPerfMode.DoubleRow
NP_F8 = ml_dtypes.float8_e4m3
NP_BF = ml_dtypes.bfloat16

SE = 16.0                     # enc fp8 scale
SW = 64.0                     # We fp8 scale
SCALE = SE * SW               # psum arrives x1024; tanh descales
NEG_BIG = -30000.0            # exp(x - 30000) == 0.0 exactly in f32
CHUNK = 3                     # s-tiles per psum tile (3 banks of 4KiB)


def build_kernel(tiles):
    """tiles: per-slot s-tile counts (max over cores), e.g. (32, 27, 20, 10)."""
    nc = bacc.Bacc("TRN2", debug=False, num_devices=N_CORES)

    enc_d = [
        nc.dram_tensor(f"enc8_{s}", [2, 128, 2 * t * 128], F8,
                       kind="ExternalInput").ap()
        for s, t in enumerate(tiles)
    ]
    cov_d = [
        nc.dram_tensor(f"cov_{s}", [2, t * 128], BF16, kind="ExternalInput").ap()
        for s, t in enumerate(tiles)
    ]
    aug_d = [
        nc.dram_tensor(f"aug_{s}", [2, H], BF16, kind="ExternalInput").ap()
        for s in range(N_SLOTS)
    ]
    we_d = nc.dram_tensor("we8", [2, 128, 2 * H], F8, kind="ExternalInput").ap()
    v_d = nc.dram_tensor("v_row", [1, H], BF16, kind="ExternalInput").ap()
    lens_d = nc.dram_tensor("lens", [N_SLOTS, 1], F32, kind="ExternalInput").ap()
    iota_d = nc.dram_tensor("iota_pm", [128, 32], F32, kind="ExternalInput").ap()
    ident_d = nc.dram_tensor("ident", [128, 128], F32, kind="ExternalInput").ap()
    out_d = [
        nc.dram_tensor(f"out_w_{s}", [t, 128], F32, kind="ExternalOutput").ap()
        for s, t in enumerate(tiles)
    ]

    with tile.TileContext(nc) as tc:
        with (
            tc.tile_pool(name="persist", bufs=1) as pp,
            tc.tile_pool(name="x", bufs=3) as xp,
            tc.tile_pool(name="scratch", bufs=2) as scrp,
            tc.tile_pool(name="small", bufs=4) as smp,
            tc.tile_pool(name="batch", bufs=3) as bp,
            tc.tile_pool(name="psum", bufs=2, space="PSUM") as psp,
            tc.tile_pool(name="psum_misc", bufs=2, space="PSUM") as psm,
        ):
            # ---- one-time setup ----
            we_sb = []
            for k2 in range(2):
                t = pp.tile([128, 2 * H], F8, tag=f"we{k2}")
                nc.sync.dma_start(t[:], we_d[k2])
                we_sb.append(t)
            enc_sb = []
            for s, nt in enumerate(tiles):
                pair = []
                for k2 in range(2):
                    t = pp.tile([128, 2 * nt * 128], F8, tag=f"enc{s}_{k2}")
                    nc.sync.dma_start(t[:], enc_d[s][k2])
                    pair.append(t)
                enc_sb.append(pair)

            vrow_sb = pp.tile([1, H], BF16, tag="vrow")
            nc.scalar.dma_start(vrow_sb[:], v_d[:, :])
            iota_sb = pp.tile([128, 32], F32, tag="iota")
            nc.scalar.dma_start(iota_sb[:], iota_d[:, :])
            ident_sb = pp.tile([128, 128], F32, tag="ident")
            nc.scalar.dma_start(ident_sb[:], ident_d[:, :])

            ones_k1 = pp.tile([1, 128], F32, tag="ones_k1")
            nc.vector.memset(ones_k1[:], 1.0)
            ones_col = pp.tile([128, 1], F32, tag="ones_col")
            nc.vector.memset(ones_col[:], 1.0)
            ones_bf = pp.tile([1, 128], BF16, tag="ones_bf")
            nc.vector.memset(ones_bf[:], 1.0)

            # dep-free matmul burst: trips the PE HAM to K=8/8 (~2.4 GHz)
            # before the real stream arrives.
            warm_f = pp.tile([128, 512], BF16, tag="warm")
            nc.vector.memset(warm_f[:], 0.5)
            for wi in range(20):
                ps_w = psm.tile([128, 512], F32, tag="mpsum")
                nc.tensor.matmul(ps_w[:], warm_f[:, 0:128], warm_f[:],
                                 start=True, stop=True)

            # v_bcast[p, o] = v_w[o]  (bf16 for the 2x DVE v-dot)
            ps_vb = psm.tile([128, H], F32, tag="mpsum")
            nc.tensor.matmul(ps_vb[:], ones_bf[:], vrow_sb[:],
                             start=True, stop=True)
            v_bcast = pp.tile([128, H], BF16, tag="v_bcast")
            nc.scalar.copy(v_bcast[:], ps_vb[:])

            state = {}

            def emit_prep(s):
                nt = tiles[s]
                cov_sb = bp.tile([2, 32 * 128], BF16, tag="cov_aug")
                nc.gpsimd.dma_start(cov_sb[:, :nt * 128], cov_d[s])
                aug_sb = bp.tile([2, H], BF16, tag="aug_rhs")
                nc.gpsimd.dma_start(aug_sb[:], aug_d[s])
                len_sb = smp.tile([1, 1], F32, tag="len_sb")
                nc.gpsimd.dma_start(len_sb[:], lens_d[s:s + 1, :])
                ps_l = psm.tile([128, 1], F32, tag="mpsum")
                nc.tensor.matmul(ps_l[:], ones_k1[:], len_sb[:],
                                 start=True, stop=True)
                l_col = smp.tile([128, 1], F32, tag="l_col")
                nc.vector.tensor_scalar(l_col[:], ps_l[:], 1.0, None, ALU.mult)
                att_pm = bp.tile([128, 32], F32, tag="att_pm")
                state[s] = dict(cov=cov_sb, aug=aug_sb, l_col=l_col,
                                att_pm=att_pm)

            def emit_chunk(s, t0, ntile):
                st8 = state[s]
                ps = psp.tile([128, CHUNK * 512], F32, tag="feat")
                for j in range(ntile):
                    t = t0 + j
                    dst = ps[:, j * 512:(j + 1) * 512]
                    for k2 in range(2):
                        lhs = (enc_sb[s][k2][:]
                               .rearrange("p (i q) -> p i q", i=2)
                               [:, :, t * 128:(t + 1) * 128])
                        rhs = we_sb[k2][:].rearrange("p (i q) -> p i q", i=2)
                        nc.tensor.matmul(dst, lhs, rhs, start=(k2 == 0),
                                         stop=False, perf_mode=DR)
                    nc.tensor.matmul(
                        dst, st8["cov"][:, t * 128:(t + 1) * 128],
                        st8["aug"][:], start=False, stop=True)
                x = xp.tile([128, CHUNK * 512], BF16, tag="x")
                nc.scalar.activation(x[:, :ntile * 512], ps[:, :ntile * 512],
                                     ACTF.Tanh, scale=1.0 / SCALE)
                for j in range(ntile):
                    t = t0 + j
                    scr = scrp.tile([128, 512], BF16, tag="vscr")
                    nc.vector.scalar_tensor_tensor(
                        scr[:], x[:, j * 512:(j + 1) * 512],
                        1.0, v_bcast[:], ALU.bypass, ALU.mult,
                        accum_out=st8["att_pm"][:, t:t + 1])

            def emit_softmax(s):
                st8 = state.pop(s)
                nt = tiles[s]
                att_pm, l_col = st8["att_pm"], st8["l_col"]
                pad01 = bp.tile([128, 32], F32, tag="pad01")
                nc.vector.tensor_scalar(pad01[:, :nt], iota_sb[:, :nt],
                                        l_col[:], None, ALU.is_ge)
                att_m = bp.tile([128, 32], F32, tag="att_m")
                nc.vector.scalar_tensor_tensor(
                    att_m[:, :nt], pad01[:, :nt], NEG_BIG, att_pm[:, :nt],
                    ALU.mult, ALU.add)
                exp_pm = bp.tile([128, 32], F32, tag="exp_pm")
                rowsum = smp.tile([128, 1], F32, tag="rowsum")
                nc.scalar.activation(exp_pm[:, :nt], att_m[:, :nt], ACTF.Exp,
                                     accum_out=rowsum[:])
                ps_d = psm.tile([1, 1], F32, tag="mpsum")
                nc.tensor.matmul(ps_d[:], rowsum[:], ones_col[:],
                                 start=True, stop=True)
                rinv = smp.tile([1, 1], F32, tag="rinv")
                nc.vector.reciprocal(rinv[:], ps_d[:])
                ps_r = psm.tile([128, 1], F32, tag="mpsum")
                nc.tensor.matmul(ps_r[:], ones_k1[:], rinv[:],
                                 start=True, stop=True)
                rinv_col = smp.tile([128, 1], F32, tag="rinv_col")
                nc.vector.tensor_scalar(rinv_col[:], ps_r[:], 1.0, None,
                                        ALU.mult)
                w_pm = bp.tile([128, 32], F32, tag="w_pm")
                nc.vector.tensor_scalar(w_pm[:, :nt], exp_pm[:, :nt],
                                        rinv_col[:], None, ALU.mult)
                ps_t = psm.tile([32, 128], F32, tag="mpsum")
                nc.tensor.transpose(ps_t[:nt, :], w_pm[:, :nt], ident_sb[:])
                w_sb = bp.tile([32, 128], F32, tag="w_sb")
                nc.vector.tensor_scalar(w_sb[:nt, :], ps_t[:nt, :], 1.0, None,
                                        ALU.mult)
                nc.sync.dma_start(out_d[s], w_sb[:nt, :])

            # chunk schedule: list of (slot, t0, ntile)
            sched = []
            for s, nt in enumerate(tiles):
                for t0 in range(0, nt, CHUNK):
                    sched.append((s, t0, min(CHUNK, nt - t0)))

            emit_prep(0)
            emit_prep(1)
            prev_slot = 0
            for (s, t0, ntile) in sched:
                if s != prev_slot:
                    # first chunk of a new slot: queue next prep, and emit
                    # the finished slot's softmax after this chunk to keep
                    # the PE stream dense.
                    if s + 1 < N_SLOTS:
                        emit_prep(s + 1)
                    emit_chunk(s, t0, ntile)
                    emit_softmax(prev_slot)
                    prev_slot = s
                else:
                    emit_chunk(s, t0, ntile)
            emit_softmax(N_SLOTS - 1)

    nc.compile()
    return nc


_NC_CACHE = {}


def _get_nc(tiles):
    key = tuple(tiles)
    if key not in _NC_CACHE:
        _NC_CACHE[key] = build_kernel(key)
    return _NC_CACHE[key]


def kernel(dec_input, enc_output, coverage_vector, text_lengths, W, b, v_w, v_b,
           _trace=False):
    dec_input = np.asarray(dec_input, np.float32)
    enc_output = np.asarray(enc_output, np.float32)
    coverage_vector = np.asarray(coverage_vector, np.float32)
    lens = np.asarray(text_lengths).astype(np.int64)
    W = np.asarray(W, np.float32)
    b = np.asarray(b, np.float32)
    v_w = np.asarray(v_w, np.float32)

    We = W[:, :H]
    Ws = W[:, H:H + D]
    Wc = W[:, H + D:]
    wc_sum = Wc.sum(axis=1)
    db = dec_input[:, 0, :] @ Ws.T + b          # [B, H] host GEMV (tiny)

    # deal batches to (core, slot) by length rank: slot s takes ranks
    # [8s, 8s+8), so the compiled per-slot cap is the max in that octet.
    order = np.argsort(-lens, kind="stable")
    assign = order.reshape(N_SLOTS, N_CORES)     # [slot, core] -> batch
    tiles = tuple(
        int(np.ceil(lens[assign[s]].max() / 128.0)) for s in range(N_SLOTS)
    )

    nc = _get_nc(tiles)

    we8 = np.ascontiguousarray(
        (We.T * SW).astype(NP_F8).reshape(2, 2, 128, H).transpose(0, 2, 1, 3)
        .reshape(2, 128, 2 * H))
    iota_pm = (np.arange(32)[None, :] * 128
               + np.arange(128)[:, None]).astype(np.float32)
    ident = np.eye(128, dtype=np.float32)
    v_bf = np.ascontiguousarray(v_w[None, :].astype(NP_BF))

    in_maps = []
    for core in range(N_CORES):
        m = {"we8": we8, "v_row": v_bf, "iota_pm": iota_pm, "ident": ident}
        lens_f = np.zeros((N_SLOTS, 1), np.float32)
        for s in range(N_SLOTS):
            bidx = int(assign[s, core])
            nt = tiles[s]
            sp = nt * 128
            lens_f[s, 0] = lens[bidx]
            e8 = (enc_output[bidx, :sp, :] * SE).astype(NP_F8)
            m[f"enc8_{s}"] = np.ascontiguousarray(
                e8.reshape(sp, 2, 2, 128).transpose(1, 3, 2, 0)
                .reshape(2, 128, 2 * sp))
            cov_aug = np.ones((2, sp), np.float32)
            cov_aug[1] = coverage_vector[bidx, :sp]
            m[f"cov_{s}"] = cov_aug.astype(NP_BF)
            aug = np.stack([db[bidx] * SCALE, wc_sum * SCALE])
            m[f"aug_{s}"] = aug.astype(NP_BF)
        m["lens"] = lens_f
        in_maps.append(m)

    res = run_bass_kernel_spmd(nc, in_maps, list(range(N_CORES)), trace=_trace)

    w = np.zeros((B, S), np.float32)
    for core in range(N_CORES):
        for s in range(N_SLOTS):
            bidx = int(assign[s, core])
            sp = tiles[s] * 128
            w[bidx, :sp] = res.results[core][f"out_w_{s}"].reshape(-1)
    c = coverage_vector + w
    if _trace:
        kernel.last_result = res
    return w, c


# revision 12
# speedup vs baseline: 1.5054x; 1.0034x over previous
"""Bass/Tile TRN2 kernel for nn_Attention_26388279067013 (v3).

Computes, for each batch row b:
    feat = enc @ We.T + dec @ Ws.T + cov[:,None] * Wc.sum(1) + b     [S, H]
    att  = tanh(feat) @ v_w                                          [S]
    att[s >= L_b] = -inf ; w = softmax(att) ; new_cov = cov + w

Key optimizations over the f32r baseline:
  - enc/We in fp8 e4m3 (x16 / x64 scaling) with DoubleRow matmuls:
    2 virtual-K=256 MMs per 128-row s-tile instead of 4 f32r MMs.
    The 1/1024 descale rides the tanh activation's free scale.
  - dec/bias/coverage rank-1 terms via one bf16 K=2 matmul per s-tile
    ([ones; cov] x [db; wc_sum], db = dec @ Ws.T + b computed on host).
  - masked positions (s >= L_b) have w == 0 exactly, so only
    ceil(L/128) s-tiles are computed. Batches are sorted by length and
    dealt round-robin to (core, slot) so the compiled per-slot tile
    counts (max over cores) stay small; host fills w=0 / c=cov for the
    skipped tail. new_cov = cov + w is a host-side add.
  - x = tanh(feat) in bf16 -> DVE scalar_tensor_tensor v-dot at 2x.

Sharding: 4 batch slots per core across 8 NeuronCores (SPMD).
"""

import sys

sys.path.insert(0, "/opt/trn_rl_repo")

import numpy as np
import ml_dtypes

import concourse.bacc as bacc
import concourse.tile as tile
import concourse.mybir as mybir
from concourse.bass_utils import run_bass_kernel_spmd

B, S, H, D = 32, 4096, 512, 256
N_CORES = 8
N_SLOTS = 4
F32 = mybir.dt.float32
BF16 = mybir.dt.bfloat16
F8 = mybir.dt.float8e4
ALU = mybir.AluOpType
ACTF = mybir.ActivationFunctionType
DR = mybir.MatmulPerfMode.DoubleRow
NP_F8 = ml_dtypes.float8_e4m3
NP_BF = ml_dtypes.bfloat16

SE = 16.0                     # enc fp8 scale
SW = 64.0                     # We fp8 scale
SCALE = SE * SW               # psum arrives x1024; tanh descales
NEG_BIG = -30000.0            # exp(x - 30000) == 0.0 exactly in f32
CHUNK = 3                     # s-tiles per psum tile (3 banks of 4KiB)


def build_kernel(tiles):
    """tiles: per-slot s-tile counts (max over cores), e.g. (32, 27, 20, 10)."""
    nc = bacc.Bacc("TRN2", debug=False, num_devices=N_CORES)

    enc_d = [
        nc.dram_tensor(f"enc8_{s}", [4, 128, t * 128], F8,
                       kind="ExternalInput").ap()
        for s, t in enumerate(tiles)
    ]
    cov_d = [
        nc.dram_tensor(f"cov_{s}", [2, t * 128], BF16, kind="ExternalInput").ap()
        for s, t in enumerate(tiles)
    ]
    aug_d = [
        nc.dram_tensor(f"aug_{s}", [2, H], BF16, kind="ExternalInput").ap()
        for s in range(N_SLOTS)
    ]
    we_d = nc.dram_tensor("we8", [4, 128, H], F8, kind="ExternalInput").ap()
    v_d = nc.dram_tensor("v_row", [1, H], BF16, kind="ExternalInput").ap()
    lens_d = nc.dram_tensor("lens", [N_SLOTS, 1], F32, kind="ExternalInput").ap()
    iota_d = nc.dram_tensor("iota_pm", [128, 32], F32, kind="ExternalInput").ap()
    ident_d = nc.dram_tensor("ident", [128, 128], F32, kind="ExternalInput").ap()
    out_d = [
        nc.dram_tensor(f"out_w_{s}", [t, 128], F32, kind="ExternalOutput").ap()
        for s, t in enumerate(tiles)
    ]

    with tile.TileContext(nc) as tc:
        with (
            tc.tile_pool(name="persist", bufs=1) as pp,
            tc.tile_pool(name="x", bufs=3) as xp,
            tc.tile_pool(name="scratch", bufs=2) as scrp,
            tc.tile_pool(name="small", bufs=4) as smp,
            tc.tile_pool(name="batch", bufs=3) as bp,
            tc.tile_pool(name="psum", bufs=2, space="PSUM") as psp,
            tc.tile_pool(name="psum_misc", bufs=2, space="PSUM") as psm,
        ):
            # ---- one-time setup ----
            we_sb = []
            for k in range(4):
                t = pp.tile([128, H], F8, tag=f"we{k}")
                nc.sync.dma_start(t[:], we_d[k])
                we_sb.append(t)
            enc_sb = []
            for s, nt in enumerate(tiles):
                quad = []
                for k in range(4):
                    t = pp.tile([128, nt * 128], F8, tag=f"enc{s}_{k}")
                    nc.sync.dma_start(t[:], enc_d[s][k])
                    quad.append(t)
                enc_sb.append(quad)

            vrow_sb = pp.tile([1, H], BF16, tag="vrow")
            nc.scalar.dma_start(vrow_sb[:], v_d[:, :])
            iota_sb = pp.tile([128, 32], F32, tag="iota")
            nc.scalar.dma_start(iota_sb[:], iota_d[:, :])
            ident_sb = pp.tile([128, 128], F32, tag="ident")
            nc.scalar.dma_start(ident_sb[:], ident_d[:, :])

            ones_k1 = pp.tile([1, 128], F32, tag="ones_k1")
            nc.vector.memset(ones_k1[:], 1.0)
            ones_col = pp.tile([128, 1], F32, tag="ones_col")
            nc.vector.memset(ones_col[:], 1.0)
            ones_bf = pp.tile([1, 128], BF16, tag="ones_bf")
            nc.vector.memset(ones_bf[:], 1.0)

            # dep-free matmul burst: trips the PE HAM to K=8/8 (~2.4 GHz)
            # before the real stream arrives.
            warm_f = pp.tile([128, 512], BF16, tag="warm")
            nc.vector.memset(warm_f[:], 0.5)
            for wi in range(20):
                ps_w = psm.tile([128, 512], F32, tag="mpsum")
                nc.tensor.matmul(ps_w[:], warm_f[:, 0:128], warm_f[:],
                                 start=True, stop=True)

            # v_bcast[p, o] = v_w[o]  (bf16 for the 2x DVE v-dot)
            ps_vb = psm.tile([128, H], F32, tag="mpsum")
            nc.tensor.matmul(ps_vb[:], ones_bf[:], vrow_sb[:],
                             start=True, stop=True)
            v_bcast = pp.tile([128, H], BF16, tag="v_bcast")
            nc.scalar.copy(v_bcast[:], ps_vb[:])

            state = {}

            def emit_prep(s):
                nt = tiles[s]
                cov_sb = bp.tile([2, 32 * 128], BF16, tag="cov_aug")
                nc.gpsimd.dma_start(cov_sb[:, :nt * 128], cov_d[s])
                aug_sb = bp.tile([2, H], BF16, tag="aug_rhs")
                nc.gpsimd.dma_start(aug_sb[:], aug_d[s])
                len_sb = smp.tile([1, 1], F32, tag="len_sb")
                nc.gpsimd.dma_start(len_sb[:], lens_d[s:s + 1, :])
                ps_l = psm.tile([128, 1], F32, tag="mpsum")
                nc.tensor.matmul(ps_l[:], ones_k1[:], len_sb[:],
                                 start=True, stop=True)
                l_col = smp.tile([128, 1], F32, tag="l_col")
                nc.vector.tensor_scalar(l_col[:], ps_l[:], 1.0, None, ALU.mult)
                att_pm = bp.tile([128, 32], F32, tag="att_pm")
                state[s] = dict(cov=cov_sb, aug=aug_sb, l_col=l_col,
                                att_pm=att_pm)

            def emit_chunk(s, t0, ntile):
                st8 = state[s]
                ps = psp.tile([128, CHUNK * 512], F32, tag="feat")
                for j in range(ntile):
                    t = t0 + j
                    dst = ps[:, j * 512:(j + 1) * 512]
                    for k in range(4):
                        nc.tensor.matmul(
                            dst, enc_sb[s][k][:, t * 128:(t + 1) * 128],
                            we_sb[k][:], start=(k == 0), stop=False)
                    nc.tensor.matmul(
                        dst, st8["cov"][:, t * 128:(t + 1) * 128],
                        st8["aug"][:], start=False, stop=True)
                x = xp.tile([128, CHUNK * 512], BF16, tag="x")
                nc.scalar.activation(x[:, :ntile * 512], ps[:, :ntile * 512],
                                     ACTF.Tanh, scale=1.0 / SCALE)
                for j in range(ntile):
                    t = t0 + j
                    scr = scrp.tile([128, 512], BF16, tag="vscr")
                    nc.vector.scalar_tensor_tensor(
                        scr[:], x[:, j * 512:(j + 1) * 512],
                        1.0, v_bcast[:], ALU.bypass, ALU.mult,
                        accum_out=st8["att_pm"][:, t:t + 1])

            def emit_softmax(s):
                st8 = state.pop(s)
                nt = tiles[s]
                att_pm, l_col = st8["att_pm"], st8["l_col"]
                pad01 = bp.tile([128, 32], F32, tag="pad01")
                nc.vector.tensor_scalar(pad01[:, :nt], iota_sb[:, :nt],
                                        l_col[:], None, ALU.is_ge)
                att_m = bp.tile([128, 32], F32, tag="att_m")
                nc.vector.scalar_tensor_tensor(
                    att_m[:, :nt], pad01[:, :nt], NEG_BIG, att_pm[:, :nt],
                    ALU.mult, ALU.add)
                exp_pm = bp.tile([128, 32], F32, tag="exp_pm")
                rowsum = smp.tile([128, 1], F32, tag="rowsum")
                nc.scalar.activation(exp_pm[:, :nt], att_m[:, :nt], ACTF.Exp,
                                     accum_out=rowsum[:])
                ps_d = psm.tile([1, 1], F32, tag="mpsum")
                nc.tensor.matmul(ps_d[:], rowsum[:], ones_col[:],
                                 start=True, stop=True)
                rinv = smp.tile([1, 1], F32, tag="rinv")
                nc.vector.reciprocal(rinv[:], ps_d[:])
                ps_r = psm.tile([128, 1], F32, tag="mpsum")
                nc.tensor.matmul(ps_r[:], ones_k1[:], rinv[:],
                                 start=True, stop=True)
                rinv_col = smp.tile([128, 1], F32, tag="rinv_col")
                nc.vector.tensor_scalar(rinv_col[:], ps_r[:], 1.0, None,
                                        ALU.mult)
                w_pm = bp.tile([128, 32], F32, tag="w_pm")
                nc.vector.tensor_scalar(w_pm[:, :nt], exp_pm[:, :nt],
                                        rinv_col[:], None, ALU.mult)
                ps_t = psm.tile([32, 128], F32, tag="mpsum")
                nc.tensor.transpose(ps_t[:nt, :], w_pm[:, :nt], ident_sb[:])
                w_sb = bp.tile([32, 128], F32, tag="w_sb")
                nc.vector.tensor_scalar(w_sb[:nt, :], ps_t[:nt, :], 1.0, None,
                                        ALU.mult)
                nc.sync.dma_start(out_d[s], w_sb[:nt, :])

            # chunk schedule: list of (slot, t0, ntile)
            sched = []
            for s, nt in enumerate(tiles):
                for t0 in range(0, nt, CHUNK):
                    sched.append((s, t0, min(CHUNK, nt - t0)))

            emit_prep(0)
            emit_prep(1)
            prev_slot = 0
            for (s, t0, ntile) in sched:
                if s != prev_slot:
                    # first chunk of a new slot: queue next prep, and emit
                    # the finished slot's softmax after this chunk to keep
                    # the PE stream dense.
                    if s + 1 < N_SLOTS:
                        emit_prep(s + 1)
                    emit_chunk(s, t0, ntile)
                    emit_softmax(prev_slot)
                    prev_slot = s
                else:
                    emit_chunk(s, t0, ntile)
            emit_softmax(N_SLOTS - 1)

    nc.compile()
    return nc


_NC_CACHE = {}


def _get_nc(tiles):
    key = tuple(tiles)
    if key not in _NC_CACHE:
        _NC_CACHE[key] = build_kernel(key)
    return _NC_CACHE[key]


def kernel(dec_input, enc_output, coverage_vector, text_lengths, W, b, v_w, v_b,
           _trace=False):
    dec_input = np.asarray(dec_input, np.float32)
    enc_output = np.asarray(enc_output, np.float32)
    coverage_vector = np.asarray(coverage_vector, np.float32)
    lens = np.asarray(text_lengths).astype(np.int64)
    W = np.asarray(W, np.float32)
    b = np.asarray(b, np.float32)
    v_w = np.asarray(v_w, np.float32)

    We = W[:, :H]
    Ws = W[:, H:H + D]
    Wc = W[:, H + D:]
    wc_sum = Wc.sum(axis=1)
    db = dec_input[:, 0, :] @ Ws.T + b          # [B, H] host GEMV (tiny)

    # deal batches to (core, slot) by length rank: slot s takes ranks
    # [8s, 8s+8), so the compiled per-slot cap is the max in that octet.
    order = np.argsort(-lens, kind="stable")
    assign = order.reshape(N_SLOTS, N_CORES)     # [slot, core] -> batch
    tiles = tuple(
        int(np.ceil(lens[assign[s]].max() / 128.0)) for s in range(N_SLOTS)
    )

    nc = _get_nc(tiles)

    we8 = np.ascontiguousarray((We.T * SW).astype(NP_F8).reshape(4, 128, H))
    iota_pm = (np.arange(32)[None, :] * 128
               + np.arange(128)[:, None]).astype(np.float32)
    ident = np.eye(128, dtype=np.float32)
    v_bf = np.ascontiguousarray(v_w[None, :].astype(NP_BF))

    in_maps = []
    for core in range(N_CORES):
        m = {"we8": we8, "v_row": v_bf, "iota_pm": iota_pm, "ident": ident}
        lens_f = np.zeros((N_SLOTS, 1), np.float32)
        for s in range(N_SLOTS):
            bidx = int(assign[s, core])
            nt = tiles[s]
            sp = nt * 128
            lens_f[s, 0] = lens[bidx]
            e8 = (enc_output[bidx, :sp, :] * SE).astype(NP_F8)
            m[f"enc8_{s}"] = np.ascontiguousarray(
                e8.reshape(sp, 4, 128).transpose(1, 2, 0))
            cov_aug = np.ones((2, sp), np.float32)
            cov_aug[1] = coverage_vector[bidx, :sp]
            m[f"cov_{s}"] = cov_aug.astype(NP_BF)
            aug = np.stack([db[bidx] * SCALE, wc_sum * SCALE])
            m[f"aug_{s}"] = aug.astype(NP_BF)
        m["lens"] = lens_f
        in_maps.append(m)

    res = run_bass_kernel_spmd(nc, in_maps, list(range(N_CORES)), trace=_trace)

    w = np.zeros((B, S), np.float32)
    for core in range(N_CORES):
        for s in range(N_SLOTS):
            bidx = int(assign[s, core])
            sp = tiles[s] * 128
            w[bidx, :sp] = res.results[core][f"out_w_{s}"].reshape(-1)
    c = coverage_vector + w
    if _trace:
        kernel.last_result = res
    return w, c


# revision 15
# speedup vs baseline: 1.9134x; 1.2710x over previous
"""Bass/Tile TRN2 kernel for nn_Attention_26388279067013 (v3).

Computes, for each batch row b:
    feat = enc @ We.T + dec @ Ws.T + cov[:,None] * Wc.sum(1) + b     [S, H]
    att  = tanh(feat) @ v_w                                          [S]
    att[s >= L_b] = -inf ; w = softmax(att) ; new_cov = cov + w

Key optimizations over the f32r baseline:
  - enc/We in fp8 e4m3 (x16 / x64 scaling) with DoubleRow matmuls:
    2 virtual-K=256 MMs per 128-row s-tile instead of 4 f32r MMs.
    The 1/1024 descale rides the tanh activation's free scale.
  - dec/bias/coverage rank-1 terms via one bf16 K=2 matmul per s-tile
    ([ones; cov] x [db; wc_sum], db = dec @ Ws.T + b computed on host).
  - masked positions (s >= L_b) have w == 0 exactly, so only
    ceil(L/128) s-tiles are computed. Batches are sorted by length and
    dealt round-robin to (core, slot) so the compiled per-slot tile
    counts (max over cores) stay small; host fills w=0 / c=cov for the
    skipped tail. new_cov = cov + w is a host-side add.
  - x = tanh(feat) in bf16 -> DVE scalar_tensor_tensor v-dot at 2x.

Sharding: 4 batch slots per core across 8 NeuronCores (SPMD).
"""

import sys

sys.path.insert(0, "/opt/trn_rl_repo")

import numpy as np
import ml_dtypes

import concourse.bacc as bacc
import concourse.tile as tile
import concourse.mybir as mybir
from concourse.bass_utils import run_bass_kernel_spmd

B, S, H, D = 32, 4096, 512, 256
N_CORES = 8
N_SLOTS = 4
F32 = mybir.dt.float32
BF16 = mybir.dt.bfloat16
F8 = mybir.dt.float8e4
ALU = mybir.AluOpType
ACTF = mybir.ActivationFunctionType
DR = mybir.MatmulPerfMode.DoubleRow
NP_F8 = ml_dtypes.float8_e4m3
NP_BF = ml_dtypes.bfloat16

SE = 16.0                     # enc fp8 scale
SW = 64.0                     # We fp8 scale
SCALE = SE * SW               # psum arrives x1024; tanh descales
NEG_BIG = -30000.0            # exp(x - 30000) == 0.0 exactly in f32
CHUNK = 3                     # s-tiles per psum tile (3 banks of 4KiB)


def build_kernel(tiles):
    """tiles: per-slot s-tile counts (max over cores), e.g. (32, 27, 20, 10)."""
    nc = bacc.Bacc("TRN2", debug=False, num_devices=N_CORES)

    enc_d = [
        nc.dram_tensor(f"enc8_{s}", [4, 128, t * 128], F8,
                       kind="ExternalInput").ap()
        for s, t in enumerate(tiles)
    ]
    cov_d = [
        nc.dram_tensor(f"cov_{s}", [2, t * 128], BF16, kind="ExternalInput").ap()
        for s, t in enumerate(tiles)
    ]
    aug_d = [
        nc.dram_tensor(f"aug_{s}", [2, H], BF16, kind="ExternalInput").ap()
        for s in range(N_SLOTS)
    ]
    we_d = nc.dram_tensor("we8", [4, 128, H], F8, kind="ExternalInput").ap()
    v_d = nc.dram_tensor("v_row", [1, H], BF16, kind="ExternalInput").ap()
    lens_d = nc.dram_tensor("lens", [N_SLOTS, 1], F32, kind="ExternalInput").ap()
    iota_d = nc.dram_tensor("iota_pm", [128, 32], F32, kind="ExternalInput").ap()
    ident_d = nc.dram_tensor("ident", [128, 128], F32, kind="ExternalInput").ap()
    out_d = [
        nc.dram_tensor(f"out_w_{s}", [t, 128], F32, kind="ExternalOutput").ap()
        for s, t in enumerate(tiles)
    ]

    with tile.TileContext(nc) as tc:
        with (
            tc.tile_pool(name="persist", bufs=1) as pp,
            tc.tile_pool(name="x", bufs=3) as xp,
            tc.tile_pool(name="scratch", bufs=2) as scrp,
            tc.tile_pool(name="small", bufs=4) as smp,
            tc.tile_pool(name="batch", bufs=3) as bp,
            tc.tile_pool(name="psum", bufs=2, space="PSUM") as psp,
            tc.tile_pool(name="psum_misc", bufs=2, space="PSUM") as psm,
        ):
            # ---- one-time setup ----
            we_sb = []
            for k in range(4):
                t = pp.tile([128, H], F8, tag=f"we{k}")
                nc.sync.dma_start(t[:], we_d[k])
                we_sb.append(t)
            enc_sb = []
            for s, nt in enumerate(tiles):
                quad = []
                for k in range(4):
                    t = pp.tile([128, nt * 128], F8, tag=f"enc{s}_{k}")
                    nc.sync.dma_start(t[:], enc_d[s][k])
                    quad.append(t)
                enc_sb.append(quad)

            # aug operands padded to K=128 (rows 2-127 zero) so the aug
            # matmul's LDWEIGHTS overlaps the preceding stream like the
            # full-K enc matmuls do (a 2-row stationary defeats the
            # weight-load pull-ahead). Double-buffered across slots.
            aug_lhs, aug_rhs = [], []
            for i in range(2):
                t = pp.tile([128, 32 * 128], BF16, tag=f"auglhs{i}")
                nc.vector.memset(t[:], 0.0)
                aug_lhs.append(t)
                t = pp.tile([128, H], BF16, tag=f"augrhs{i}")
                nc.vector.memset(t[:], 0.0)
                aug_rhs.append(t)

            vrow_sb = pp.tile([1, H], BF16, tag="vrow")
            nc.scalar.dma_start(vrow_sb[:], v_d[:, :])
            iota_sb = pp.tile([128, 32], F32, tag="iota")
            nc.scalar.dma_start(iota_sb[:], iota_d[:, :])
            ident_sb = pp.tile([128, 128], F32, tag="ident")
            nc.scalar.dma_start(ident_sb[:], ident_d[:, :])

            ones_k1 = pp.tile([1, 128], F32, tag="ones_k1")
            nc.vector.memset(ones_k1[:], 1.0)
            ones_col = pp.tile([128, 1], F32, tag="ones_col")
            nc.vector.memset(ones_col[:], 1.0)
            ones_bf = pp.tile([1, 128], BF16, tag="ones_bf")
            nc.vector.memset(ones_bf[:], 1.0)

            # dep-free matmul burst: trips the PE HAM to K=8/8 (~2.4 GHz)
            # before the real stream arrives.
            warm_f = pp.tile([128, 512], BF16, tag="warm")
            nc.vector.memset(warm_f[:], 0.5)
            for wi in range(20):
                ps_w = psm.tile([128, 512], F32, tag="mpsum")
                nc.tensor.matmul(ps_w[:], warm_f[:, 0:128], warm_f[:],
                                 start=True, stop=True)

            # v_bcast[p, o] = v_w[o]  (bf16 for the 2x DVE v-dot)
            ps_vb = psm.tile([128, H], F32, tag="mpsum")
            nc.tensor.matmul(ps_vb[:], ones_bf[:], vrow_sb[:],
                             start=True, stop=True)
            v_bcast = pp.tile([128, H], BF16, tag="v_bcast")
            nc.scalar.copy(v_bcast[:], ps_vb[:])

            state = {}

            def emit_prep(s):
                nt = tiles[s]
                cov_sb = aug_lhs[s % 2]
                nc.gpsimd.dma_start(cov_sb[0:2, :nt * 128], cov_d[s])
                aug_sb = aug_rhs[s % 2]
                nc.gpsimd.dma_start(aug_sb[0:2, :], aug_d[s])
                len_sb = smp.tile([1, 1], F32, tag="len_sb")
                nc.gpsimd.dma_start(len_sb[:], lens_d[s:s + 1, :])
                ps_l = psm.tile([128, 1], F32, tag="mpsum")
                nc.tensor.matmul(ps_l[:], ones_k1[:], len_sb[:],
                                 start=True, stop=True)
                l_col = smp.tile([128, 1], F32, tag="l_col")
                nc.vector.tensor_scalar(l_col[:], ps_l[:], 1.0, None, ALU.mult)
                att_pm = bp.tile([128, 32], F32, tag="att_pm")
                state[s] = dict(cov=cov_sb, aug=aug_sb, l_col=l_col,
                                att_pm=att_pm)

            def emit_chunk(s, t0, ntile):
                st8 = state[s]
                ps = psp.tile([128, CHUNK * 512], F32, tag="feat")
                for j in range(ntile):
                    t = t0 + j
                    dst = ps[:, j * 512:(j + 1) * 512]
                    for k in range(4):
                        nc.tensor.matmul(
                            dst, enc_sb[s][k][:, t * 128:(t + 1) * 128],
                            we_sb[k][:], start=(k == 0), stop=False)
                    nc.tensor.matmul(
                        dst, st8["cov"][:, t * 128:(t + 1) * 128],
                        st8["aug"][:, :], start=False, stop=True)
                x = xp.tile([128, CHUNK * 512], BF16, tag="x")
                nc.scalar.activation(x[:, :ntile * 512], ps[:, :ntile * 512],
                                     ACTF.Tanh, scale=1.0 / SCALE)
                for j in range(ntile):
                    t = t0 + j
                    scr = scrp.tile([128, 512], BF16, tag="vscr")
                    nc.vector.scalar_tensor_tensor(
                        scr[:], x[:, j * 512:(j + 1) * 512],
                        1.0, v_bcast[:], ALU.bypass, ALU.mult,
                        accum_out=st8["att_pm"][:, t:t + 1])

            def emit_softmax(s):
                st8 = state.pop(s)
                nt = tiles[s]
                att_pm, l_col = st8["att_pm"], st8["l_col"]
                pad01 = bp.tile([128, 32], F32, tag="pad01")
                nc.vector.tensor_scalar(pad01[:, :nt], iota_sb[:, :nt],
                                        l_col[:], None, ALU.is_ge)
                att_m = bp.tile([128, 32], F32, tag="att_m")
                nc.vector.scalar_tensor_tensor(
                    att_m[:, :nt], pad01[:, :nt], NEG_BIG, att_pm[:, :nt],
                    ALU.mult, ALU.add)
                exp_pm = bp.tile([128, 32], F32, tag="exp_pm")
                rowsum = smp.tile([128, 1], F32, tag="rowsum")
                nc.scalar.activation(exp_pm[:, :nt], att_m[:, :nt], ACTF.Exp,
                                     accum_out=rowsum[:])
                ps_d = psm.tile([1, 1], F32, tag="mpsum")
                nc.tensor.matmul(ps_d[:], rowsum[:], ones_col[:],
                                 start=True, stop=True)
                rinv = smp.tile([1, 1], F32, tag="rinv")
                nc.vector.reciprocal(rinv[:], ps_d[:])
                ps_r = psm.tile([128, 1], F32, tag="mpsum")
                nc.tensor.matmul(ps_r[:], ones_k1[:], rinv[:],
                                 start=True, stop=True)
                rinv_col = smp.tile([128, 1], F32, tag="rinv_col")
                nc.vector.tensor_scalar(rinv_col[:], ps_r[:], 1.0, None,
                                        ALU.mult)
                w_pm = bp.tile([128, 32], F32, tag="w_pm")
                nc.vector.tensor_scalar(w_pm[:, :nt], exp_pm[:, :nt],
                                        rinv_col[:], None, ALU.mult)
                ps_t = psm.tile([32, 128], F32, tag="mpsum")
                nc.tensor.transpose(ps_t[:nt, :], w_pm[:, :nt], ident_sb[:])
                w_sb = bp.tile([32, 128], F32, tag="w_sb")
                nc.vector.tensor_scalar(w_sb[:nt, :], ps_t[:nt, :], 1.0, None,
                                        ALU.mult)
                nc.sync.dma_start(out_d[s], w_sb[:nt, :])

            # chunk schedule: list of (slot, t0, ntile)
            sched = []
            for s, nt in enumerate(tiles):
                for t0 in range(0, nt, CHUNK):
                    sched.append((s, t0, min(CHUNK, nt - t0)))

            emit_prep(0)
            emit_prep(1)
            prev_slot = 0
            for (s, t0, ntile) in sched:
                if s != prev_slot:
                    # first chunk of a new slot: queue next prep, and emit
                    # the finished slot's softmax after this chunk to keep
                    # the PE stream dense.
                    if s + 1 < N_SLOTS:
                        emit_prep(s + 1)
                    emit_chunk(s, t0, ntile)
                    emit_softmax(prev_slot)
                    prev_slot = s
                else:
                    emit_chunk(s, t0, ntile)
            emit_softmax(N_SLOTS - 1)

    nc.compile()
    return nc


_NC_CACHE = {}


def _get_nc(tiles):
    key = tuple(tiles)
    if key not in _NC_CACHE:
        _NC_CACHE[key] = build_kernel(key)
    return _NC_CACHE[key]


def kernel(dec_input, enc_output, coverage_vector, text_lengths, W, b, v_w, v_b,
           _trace=False):
    dec_input = np.asarray(dec_input, np.float32)
    enc_output = np.asarray(enc_output, np.float32)
    coverage_vector = np.asarray(coverage_vector, np.float32)
    lens = np.asarray(text_lengths).astype(np.int64)
    W = np.asarray(W, np.float32)
    b = np.asarray(b, np.float32)
    v_w = np.asarray(v_w, np.float32)

    We = W[:, :H]
    Ws = W[:, H:H + D]
    Wc = W[:, H + D:]
    wc_sum = Wc.sum(axis=1)
    db = dec_input[:, 0, :] @ Ws.T + b          # [B, H] host GEMV (tiny)

    # deal batches to (core, slot) by length rank: slot s takes ranks
    # [8s, 8s+8), so the compiled per-slot cap is the max in that octet.
    order = np.argsort(-lens, kind="stable")
    assign = order.reshape(N_SLOTS, N_CORES)     # [slot, core] -> batch
    tiles = tuple(
        int(np.ceil(lens[assign[s]].max() / 128.0)) for s in range(N_SLOTS)
    )

    nc = _get_nc(tiles)

    we8 = np.ascontiguousarray((We.T * SW).astype(NP_F8).reshape(4, 128, H))
    iota_pm = (np.arange(32)[None, :] * 128
               + np.arange(128)[:, None]).astype(np.float32)
    ident = np.eye(128, dtype=np.float32)
    v_bf = np.ascontiguousarray(v_w[None, :].astype(NP_BF))

    in_maps = []
    for core in range(N_CORES):
        m = {"we8": we8, "v_row": v_bf, "iota_pm": iota_pm, "ident": ident}
        lens_f = np.zeros((N_SLOTS, 1), np.float32)
        for s in range(N_SLOTS):
            bidx = int(assign[s, core])
            nt = tiles[s]
            sp = nt * 128
            lens_f[s, 0] = lens[bidx]
            e8 = (enc_output[bidx, :sp, :] * SE).astype(NP_F8)
            m[f"enc8_{s}"] = np.ascontiguousarray(
                e8.reshape(sp, 4, 128).transpose(1, 2, 0))
            cov_aug = np.ones((2, sp), np.float32)
            cov_aug[1] = coverage_vector[bidx, :sp]
            m[f"cov_{s}"] = cov_aug.astype(NP_BF)
            aug = np.stack([db[bidx] * SCALE, wc_sum * SCALE])
            m[f"aug_{s}"] = aug.astype(NP_BF)
        m["lens"] = lens_f
        in_maps.append(m)

    res = run_bass_kernel_spmd(nc, in_maps, list(range(N_CORES)), trace=_trace)

    w = np.zeros((B, S), np.float32)
    for core in range(N_CORES):
        for s in range(N_SLOTS):
            bidx = int(assign[s, core])
            sp = tiles[s] * 128
            w[bidx, :sp] = res.results[core][f"out_w_{s}"].reshape(-1)
    c = coverage_vector + w
    if _trace:
        kernel.last_result = res
    return w, c


# revision 23
# speedup vs baseline: 2.0043x; 1.0475x over previous
"""Bass/Tile TRN2 kernel for nn_Attention_26388279067013 (v3).

Computes, for each batch row b:
    feat = enc @ We.T + dec @ Ws.T + cov[:,None] * Wc.sum(1) + b     [S, H]
    att  = tanh(feat) @ v_w                                          [S]
    att[s >= L_b] = -inf ; w = softmax(att) ; new_cov = cov + w

Key optimizations over the f32r baseline:
  - enc/We in fp8 e4m3 (x16 / x64 scaling) with DoubleRow matmuls:
    2 virtual-K=256 MMs per 128-row s-tile instead of 4 f32r MMs.
    The 1/1024 descale rides the tanh activation's free scale.
  - dec/bias/coverage rank-1 terms via one bf16 K=2 matmul per s-tile
    ([ones; cov] x [db; wc_sum], db = dec @ Ws.T + b computed on host).
  - masked positions (s >= L_b) have w == 0 exactly, so only
    ceil(L/128) s-tiles are computed. Batches are sorted by length and
    dealt round-robin to (core, slot) so the compiled per-slot tile
    counts (max over cores) stay small; host fills w=0 / c=cov for the
    skipped tail. new_cov = cov + w is a host-side add.
  - x = tanh(feat) in bf16 -> DVE scalar_tensor_tensor v-dot at 2x.

Sharding: 4 batch slots per core across 8 NeuronCores (SPMD).
"""

import sys

sys.path.insert(0, "/opt/trn_rl_repo")

import numpy as np
import ml_dtypes

import concourse.bacc as bacc
import concourse.tile as tile
import concourse.mybir as mybir
from concourse.bass_utils import run_bass_kernel_spmd

B, S, H, D = 32, 4096, 512, 256
N_CORES = 8
N_SLOTS = 4
F32 = mybir.dt.float32
BF16 = mybir.dt.bfloat16
F8 = mybir.dt.float8e4
ALU = mybir.AluOpType
ACTF = mybir.ActivationFunctionType
DR = mybir.MatmulPerfMode.DoubleRow
NP_F8 = ml_dtypes.float8_e4m3
NP_BF = ml_dtypes.bfloat16

SE = 16.0                     # enc fp8 scale
SW = 64.0                     # We fp8 scale
SCALE = SE * SW               # psum arrives x1024; tanh descales
NEG_BIG = -30000.0            # exp(x - 30000) == 0.0 exactly in f32
CHUNK = 3                     # s-tiles per psum tile (3 banks of 4KiB)


def build_kernel(tiles):
    """tiles: per-slot s-tile counts (max over cores), e.g. (32, 27, 20, 10)."""
    nc = bacc.Bacc("TRN2", debug=False, num_devices=N_CORES)

    enc_d = [
        nc.dram_tensor(f"enc8_{s}", [4, 128, t * 128], F8,
                       kind="ExternalInput").ap()
        for s, t in enumerate(tiles)
    ]
    cov_d = [
        nc.dram_tensor(f"cov_{s}", [2, t * 128], BF16, kind="ExternalInput").ap()
        for s, t in enumerate(tiles)
    ]
    aug_d = [
        nc.dram_tensor(f"aug_{s}", [2, H], BF16, kind="ExternalInput").ap()
        for s in range(N_SLOTS)
    ]
    we_d = nc.dram_tensor("we8", [4, 128, H], F8, kind="ExternalInput").ap()
    v_d = nc.dram_tensor("v_row", [1, H], BF16, kind="ExternalInput").ap()
    lens_d = nc.dram_tensor("lens", [N_SLOTS, 1], F32, kind="ExternalInput").ap()
    iota_d = nc.dram_tensor("iota_pm", [128, 32], F32, kind="ExternalInput").ap()
    ident_d = nc.dram_tensor("ident", [128, 128], F32, kind="ExternalInput").ap()
    out_d = [
        nc.dram_tensor(f"out_w_{s}", [t, 128], F32, kind="ExternalOutput").ap()
        for s, t in enumerate(tiles)
    ]

    with tile.TileContext(nc) as tc:
        with (
            tc.tile_pool(name="persist", bufs=1) as pp,
            tc.tile_pool(name="x", bufs=3) as xp,
            tc.tile_pool(name="scratch", bufs=2) as scrp,
            tc.tile_pool(name="small", bufs=4) as smp,
            tc.tile_pool(name="batch", bufs=3) as bp,
            tc.tile_pool(name="psum", bufs=2, space="PSUM") as psp,
            tc.tile_pool(name="psum_misc", bufs=2, space="PSUM") as psm,
        ):
            # ---- one-time setup ----
            # warmup operand first so the HAM-warming burst starts at t~0
            warm_f = pp.tile([128, 512], BF16, tag="warm")
            nc.vector.memset(warm_f[:], 0.5)

            we_sb = []
            for k in range(4):
                t = pp.tile([128, H], F8, tag=f"we{k}")
                nc.scalar.dma_start(t[:], we_d[k])
                we_sb.append(t)
            # slot 0 head (first 2 chunks) lands fast so the real stream
            # starts ~6us in; tails and later slots follow.
            enc_sb = []
            for s, nt in enumerate(tiles):
                quad = []
                for k in range(4):
                    t = pp.tile([128, nt * 128], F8, tag=f"enc{s}_{k}")
                    quad.append(t)
                enc_sb.append(quad)
            head = min(2 * CHUNK, tiles[0]) * 128
            for k in range(4):
                nc.sync.dma_start(enc_sb[0][k][:, :head], enc_d[0][k][:, :head])
            for k in range(4):
                if tiles[0] * 128 > head:
                    nc.sync.dma_start(enc_sb[0][k][:, head:],
                                      enc_d[0][k][:, head:])
            for s in (1, 2, 3):
                for k in range(4):
                    nc.sync.dma_start(enc_sb[s][k][:], enc_d[s][k])

            # aug operands padded to K=128 (rows 2-127 zero) so the aug
            # matmul's LDWEIGHTS overlaps the preceding stream like the
            # full-K enc matmuls do (a 2-row stationary defeats the
            # weight-load pull-ahead). Double-buffered across slots.
            aug_lhs, aug_rhs = [], []
            for i in range(2):
                t = pp.tile([128, 32 * 128], BF16, tag=f"auglhs{i}")
                nc.vector.memset(t[:], 0.0)
                aug_lhs.append(t)
                t = pp.tile([128, H], BF16, tag=f"augrhs{i}")
                nc.vector.memset(t[:], 0.0)
                aug_rhs.append(t)

            vrow_sb = pp.tile([1, H], BF16, tag="vrow")
            nc.scalar.dma_start(vrow_sb[:], v_d[:, :])
            iota_sb = pp.tile([128, 32], F32, tag="iota")
            nc.scalar.dma_start(iota_sb[:], iota_d[:, :])
            ident_sb = pp.tile([128, 128], F32, tag="ident")
            nc.scalar.dma_start(ident_sb[:], ident_d[:, :])

            ones_k1 = pp.tile([1, 128], F32, tag="ones_k1")
            nc.vector.memset(ones_k1[:], 1.0)
            ones_col = pp.tile([128, 1], F32, tag="ones_col")
            nc.vector.memset(ones_col[:], 1.0)
            ones_bf = pp.tile([1, 128], BF16, tag="ones_bf")
            nc.vector.memset(ones_bf[:], 1.0)

            # dep-free matmul burst: trips the PE HAM to K=8/8 (~2.4 GHz)
            # before the real stream arrives.
            for wi in range(20):
                ps_w = psm.tile([128, 512], F32, tag="mpsum")
                nc.tensor.matmul(ps_w[:], warm_f[:, 0:128], warm_f[:],
                                 start=True, stop=True)

            # v_bcast[p, o] = v_w[o]  (bf16 for the 2x DVE v-dot)
            ps_vb = psm.tile([128, H], F32, tag="mpsum")
            nc.tensor.matmul(ps_vb[:], ones_bf[:], vrow_sb[:],
                             start=True, stop=True)
            v_bcast = pp.tile([128, H], BF16, tag="v_bcast")
            nc.scalar.copy(v_bcast[:], ps_vb[:])

            state = {}

            def emit_prep(s):
                nt = tiles[s]
                cov_sb = aug_lhs[s % 2]
                nc.gpsimd.dma_start(cov_sb[0:2, :nt * 128], cov_d[s])
                aug_sb = aug_rhs[s % 2]
                nc.gpsimd.dma_start(aug_sb[0:2, :], aug_d[s])
                len_sb = smp.tile([1, 1], F32, tag="len_sb")
                nc.gpsimd.dma_start(len_sb[:], lens_d[s:s + 1, :])
                ps_l = psm.tile([128, 1], F32, tag="mpsum")
                nc.tensor.matmul(ps_l[:], ones_k1[:], len_sb[:],
                                 start=True, stop=True)
                l_col = smp.tile([128, 1], F32, tag="l_col")
                nc.vector.tensor_scalar(l_col[:], ps_l[:], 1.0, None, ALU.mult)
                att_pm = bp.tile([128, 32], F32, tag="att_pm")
                state[s] = dict(cov=cov_sb, aug=aug_sb, l_col=l_col,
                                att_pm=att_pm)

            def emit_chunk(s, t0, ntile):
                st8 = state[s]
                ps = psp.tile([128, CHUNK * 512], F32, tag="feat")
                for j in range(ntile):
                    t = t0 + j
                    dst = ps[:, j * 512:(j + 1) * 512]
                    for k in range(4):
                        nc.tensor.matmul(
                            dst, enc_sb[s][k][:, t * 128:(t + 1) * 128],
                            we_sb[k][:], start=(k == 0), stop=False)
                    nc.tensor.matmul(
                        dst, st8["cov"][:, t * 128:(t + 1) * 128],
                        st8["aug"][:, :], start=False, stop=True)
                x = xp.tile([128, CHUNK * 512], BF16, tag="x")
                nc.scalar.activation(x[:, :ntile * 512], ps[:, :ntile * 512],
                                     ACTF.Tanh, scale=1.0 / SCALE)
                for j in range(ntile):
                    t = t0 + j
                    scr = scrp.tile([128, 512], BF16, tag="vscr")
                    nc.vector.scalar_tensor_tensor(
                        scr[:], x[:, j * 512:(j + 1) * 512],
                        1.0, v_bcast[:], ALU.bypass, ALU.mult,
                        accum_out=st8["att_pm"][:, t:t + 1])

            def emit_softmax_a(s):
                st8 = state[s]
                nt = tiles[s]
                att_pm, l_col = st8["att_pm"], st8["l_col"]
                pad01 = bp.tile([128, 32], F32, tag="pad01")
                nc.vector.tensor_scalar(pad01[:, :nt], iota_sb[:, :nt],
                                        l_col[:], None, ALU.is_ge)
                att_m = bp.tile([128, 32], F32, tag="att_m")
                nc.vector.scalar_tensor_tensor(
                    att_m[:, :nt], pad01[:, :nt], NEG_BIG, att_pm[:, :nt],
                    ALU.mult, ALU.add)
                exp_pm = bp.tile([128, 32], F32, tag="exp_pm")
                rowsum = smp.tile([128, 1], F32, tag="rowsum")
                nc.scalar.activation(exp_pm[:, :nt], att_m[:, :nt], ACTF.Exp,
                                     accum_out=rowsum[:])
                st8["exp_pm"] = exp_pm
                st8["rowsum"] = rowsum

            def emit_softmax_b(s):
                st8 = state.pop(s)
                nt = tiles[s]
                exp_pm, rowsum = st8["exp_pm"], st8["rowsum"]
                ps_d = psm.tile([1, 1], F32, tag="mpsum")
                nc.tensor.matmul(ps_d[:], rowsum[:], ones_col[:],
                                 start=True, stop=True)
                rinv = smp.tile([1, 1], F32, tag="rinv")
                nc.vector.reciprocal(rinv[:], ps_d[:])
                ps_r = psm.tile([128, 1], F32, tag="mpsum")
                nc.tensor.matmul(ps_r[:], ones_k1[:], rinv[:],
                                 start=True, stop=True)
                rinv_col = smp.tile([128, 1], F32, tag="rinv_col")
                nc.vector.tensor_scalar(rinv_col[:], ps_r[:], 1.0, None,
                                        ALU.mult)
                w_pm = bp.tile([128, 32], F32, tag="w_pm")
                nc.vector.tensor_scalar(w_pm[:, :nt], exp_pm[:, :nt],
                                        rinv_col[:], None, ALU.mult)
                ps_t = psm.tile([32, 128], F32, tag="mpsum")
                nc.tensor.transpose(ps_t[:nt, :], w_pm[:, :nt], ident_sb[:])
                w_sb = bp.tile([32, 128], F32, tag="w_sb")
                nc.vector.tensor_scalar(w_sb[:nt, :], ps_t[:nt, :], 1.0, None,
                                        ALU.mult)
                nc.sync.dma_start(out_d[s], w_sb[:nt, :])

            # chunk schedule: list of (slot, t0, ntile)
            sched = []
            for s, nt in enumerate(tiles):
                for t0 in range(0, nt, CHUNK):
                    sched.append((s, t0, min(CHUNK, nt - t0)))

            emit_prep(0)
            emit_prep(1)
            # softmax is split: stage a (DVE/ACT) right after the slot's
            # last chunk; stage b (PE-dependent chain) two chunks later so
            # the strict PE queue never waits on the exp.
            pend_a, pend_b = [], []
            prev_slot = 0
            for ci, (s, t0, ntile) in enumerate(sched):
                if s != prev_slot:
                    if s + 1 < N_SLOTS:
                        emit_prep(s + 1)
                    pend_a.append((ci, prev_slot))
                    prev_slot = s
                emit_chunk(s, t0, ntile)
                if pend_a and pend_a[0][0] == ci:
                    _, ps_ = pend_a.pop(0)
                    emit_softmax_a(ps_)
                    pend_b.append((ci + 2, ps_))
                if pend_b and pend_b[0][0] == ci:
                    _, ps_ = pend_b.pop(0)
                    emit_softmax_b(ps_)
            for _, ps_ in pend_b:
                emit_softmax_b(ps_)
            emit_softmax_a(N_SLOTS - 1)
            emit_softmax_b(N_SLOTS - 1)

    nc.compile()
    return nc


_NC_CACHE = {}


def _get_nc(tiles):
    key = tuple(tiles)
    if key not in _NC_CACHE:
        _NC_CACHE[key] = build_kernel(key)
    return _NC_CACHE[key]


def kernel(dec_input, enc_output, coverage_vector, text_lengths, W, b, v_w, v_b,
           _trace=False):
    dec_input = np.asarray(dec_input, np.float32)
    enc_output = np.asarray(enc_output, np.float32)
    coverage_vector = np.asarray(coverage_vector, np.float32)
    lens = np.asarray(text_lengths).astype(np.int64)
    W = np.asarray(W, np.float32)
    b = np.asarray(b, np.float32)
    v_w = np.asarray(v_w, np.float32)

    We = W[:, :H]
    Ws = W[:, H:H + D]
    Wc = W[:, H + D:]
    wc_sum = Wc.sum(axis=1)
    db = dec_input[:, 0, :] @ Ws.T + b          # [B, H] host GEMV (tiny)

    # deal batches to (core, slot) by length rank: slot s takes ranks
    # [8s, 8s+8), so the compiled per-slot cap is the max in that octet.
    order = np.argsort(-lens, kind="stable")
    assign = order.reshape(N_SLOTS, N_CORES)     # [slot, core] -> batch
    tiles = tuple(
        int(np.ceil(lens[assign[s]].max() / 128.0)) for s in range(N_SLOTS)
    )

    nc = _get_nc(tiles)

    we8 = np.ascontiguousarray((We.T * SW).astype(NP_F8).reshape(4, 128, H))
    iota_pm = (np.arange(32)[None, :] * 128
               + np.arange(128)[:, None]).astype(np.float32)
    ident = np.eye(128, dtype=np.float32)
    v_bf = np.ascontiguousarray(v_w[None, :].astype(NP_BF))

    in_maps = []
    for core in range(N_CORES):
        m = {"we8": we8, "v_row": v_bf, "iota_pm": iota_pm, "ident": ident}
        lens_f = np.zeros((N_SLOTS, 1), np.float32)
        for s in range(N_SLOTS):
            bidx = int(assign[s, core])
            nt = tiles[s]
            sp = nt * 128
            lens_f[s, 0] = lens[bidx]
            e8 = (enc_output[bidx, :sp, :] * SE).astype(NP_F8)
            m[f"enc8_{s}"] = np.ascontiguousarray(
                e8.reshape(sp, 4, 128).transpose(1, 2, 0))
            cov_aug = np.ones((2, sp), np.float32)
            cov_aug[1] = coverage_vector[bidx, :sp]
            m[f"cov_{s}"] = cov_aug.astype(NP_BF)
            aug = np.stack([db[bidx] * SCALE, wc_sum * SCALE])
            m[f"aug_{s}"] = aug.astype(NP_BF)
        m["lens"] = lens_f
        in_maps.append(m)

    res = run_bass_kernel_spmd(nc, in_maps, list(range(N_CORES)), trace=_trace)

    w = np.zeros((B, S), np.float32)
    for core in range(N_CORES):
        for s in range(N_SLOTS):
            bidx = int(assign[s, core])
            sp = tiles[s] * 128
            w[bidx, :sp] = res.results[core][f"out_w_{s}"].reshape(-1)
    c = coverage_vector + w
    if _trace:
        kernel.last_result = res
    return w, c


# revision 32
# speedup vs baseline: 2.2870x; 1.1410x over previous
"""Bass/Tile TRN2 kernel for nn_Attention_26388279067013 (v3).

Computes, for each batch row b:
    feat = enc @ We.T + dec @ Ws.T + cov[:,None] * Wc.sum(1) + b     [S, H]
    att  = tanh(feat) @ v_w                                          [S]
    att[s >= L_b] = -inf ; w = softmax(att) ; new_cov = cov + w

Key optimizations over the f32r baseline:
  - enc/We in fp8 e4m3 (x16 / x64 scaling) with DoubleRow matmuls:
    2 virtual-K=256 MMs per 128-row s-tile instead of 4 f32r MMs.
    The 1/1024 descale rides the tanh activation's free scale.
  - dec/bias/coverage rank-1 terms via one bf16 K=2 matmul per s-tile
    ([ones; cov] x [db; wc_sum], db = dec @ Ws.T + b computed on host).
  - masked positions (s >= L_b) have w == 0 exactly, so only
    ceil(L/128) s-tiles are computed. Batches are sorted by length and
    dealt round-robin to (core, slot) so the compiled per-slot tile
    counts (max over cores) stay small; host fills w=0 / c=cov for the
    skipped tail. new_cov = cov + w is a host-side add.
  - x = tanh(feat) in bf16 -> DVE scalar_tensor_tensor v-dot at 2x.

Sharding: 4 batch slots per core across 8 NeuronCores (SPMD).
"""

import sys

sys.path.insert(0, "/opt/trn_rl_repo")

import numpy as np
import ml_dtypes

import concourse.bacc as bacc
import concourse.tile as tile
import concourse.mybir as mybir
from concourse.bass_utils import run_bass_kernel_spmd

B, S, H, D = 32, 4096, 512, 256
N_CORES = 8
N_SLOTS = 4
F32 = mybir.dt.float32
BF16 = mybir.dt.bfloat16
F8 = mybir.dt.float8e4
ALU = mybir.AluOpType
ACTF = mybir.ActivationFunctionType
DR = mybir.MatmulPerfMode.DoubleRow
NP_F8 = ml_dtypes.float8_e4m3
NP_BF = ml_dtypes.bfloat16

SE = 16.0                     # enc fp8 scale
SW = 64.0                     # We fp8 scale
SCALE = SE * SW               # psum arrives x1024; tanh descales
NEG_BIG = -30000.0            # exp(x - 30000) == 0.0 exactly in f32
CHUNK = 3                     # s-tiles per psum tile (3 banks of 4KiB)
DR_EVERY = 2                  # every DR_EVERY-th s-tile uses DoubleRow
                              # (DR is invisible to the PE clock governor,
                              #  so plain-fp8 tiles must dominate the duty
                              #  cycle to keep the array at 2.4 GHz); 0 = off


def build_kernel(tiles):
    """tiles: per-slot s-tile counts (max over cores), e.g. (32, 27, 20, 10)."""
    nc = bacc.Bacc("TRN2", debug=False, num_devices=N_CORES)

    enc_d = [
        nc.dram_tensor(f"enc8_{s}", [128, 4, t * 128], F8,
                       kind="ExternalInput").ap()
        for s, t in enumerate(tiles)
    ]
    cov_d = [
        nc.dram_tensor(f"cov_{s}", [2, t * 128], BF16, kind="ExternalInput").ap()
        for s, t in enumerate(tiles)
    ]
    aug_d = [
        nc.dram_tensor(f"aug_{s}", [2, H], BF16, kind="ExternalInput").ap()
        for s in range(N_SLOTS)
    ]
    we_d = nc.dram_tensor("we8", [128, 4 * H], F8, kind="ExternalInput").ap()
    v_d = nc.dram_tensor("v_row", [1, H], BF16, kind="ExternalInput").ap()
    lens_d = nc.dram_tensor("lens", [N_SLOTS, 1], F32, kind="ExternalInput").ap()
    iota_d = nc.dram_tensor("iota_pm", [128, 32], F32, kind="ExternalInput").ap()
    ident_d = nc.dram_tensor("ident", [128, 128], F32, kind="ExternalInput").ap()
    out_d = [
        nc.dram_tensor(f"out_w_{s}", [t, 128], F32, kind="ExternalOutput").ap()
        for s, t in enumerate(tiles)
    ]

    with tile.TileContext(nc) as tc:
        with (
            tc.tile_pool(name="persist", bufs=1) as pp,
            tc.tile_pool(name="x", bufs=3) as xp,
            tc.tile_pool(name="scratch", bufs=2) as scrp,
            tc.tile_pool(name="small", bufs=4) as smp,
            tc.tile_pool(name="batch", bufs=3) as bp,
            tc.tile_pool(name="psum", bufs=2, space="PSUM") as psp,
            tc.tile_pool(name="psum_misc", bufs=2, space="PSUM") as psm,
        ):
            # ---- one-time setup ----
            we_t = pp.tile([128, 4 * H], F8, tag="we8")
            nc.scalar.dma_start(we_t[:], we_d[:, :])
            # [p, k, s] layout: plain matmuls slice one k, DoubleRow
            # matmuls slice a k-pair. Slot-0 head (2 chunks) lands first
            # so the stream starts early.
            enc_sb = []
            for s, nt in enumerate(tiles):
                enc_t = pp.tile([128, 4 * nt * 128], F8, tag=f"enc{s}")
                enc_sb.append(enc_t)
            head = min(2 * CHUNK, tiles[0]) * 128
            e0 = enc_sb[0][:].rearrange("p (k q) -> p k q", k=4)
            nc.sync.dma_start(e0[:, :, :head], enc_d[0][:, :, :head])
            if tiles[0] * 128 > head:
                nc.sync.dma_start(e0[:, :, head:], enc_d[0][:, :, head:])
            for s in (1, 2, 3):
                nc.sync.dma_start(
                    enc_sb[s][:],
                    enc_d[s].rearrange("p k q -> p (k q)"))

            # aug operands padded to K=128 (rows 2-127 zero) so the aug
            # matmul's LDWEIGHTS overlaps the preceding stream like the
            # full-K enc matmuls do (a 2-row stationary defeats the
            # weight-load pull-ahead). Double-buffered across slots.
            aug_lhs, aug_rhs = [], []
            for i in range(2):
                t = pp.tile([128, 32 * 128], BF16, tag=f"auglhs{i}")
                nc.vector.memset(t[:], 0.0)
                aug_lhs.append(t)
                t = pp.tile([128, H], BF16, tag=f"augrhs{i}")
                nc.vector.memset(t[:], 0.0)
                aug_rhs.append(t)

            vrow_sb = pp.tile([1, H], BF16, tag="vrow")
            nc.scalar.dma_start(vrow_sb[:], v_d[:, :])
            iota_sb = pp.tile([128, 32], F32, tag="iota")
            nc.scalar.dma_start(iota_sb[:], iota_d[:, :])
            ident_sb = pp.tile([128, 128], F32, tag="ident")
            nc.scalar.dma_start(ident_sb[:], ident_d[:, :])

            ones_k1 = pp.tile([1, 128], F32, tag="ones_k1")
            nc.vector.memset(ones_k1[:], 1.0)
            ones_col = pp.tile([128, 1], F32, tag="ones_col")
            nc.vector.memset(ones_col[:], 1.0)
            ones_bf = pp.tile([1, 128], BF16, tag="ones_bf")
            nc.vector.memset(ones_bf[:], 1.0)

            # v_bcast[p, o] = v_w[o]  (bf16 for the 2x DVE v-dot)
            ps_vb = psm.tile([128, H], F32, tag="mpsum")
            nc.tensor.matmul(ps_vb[:], ones_bf[:], vrow_sb[:],
                             start=True, stop=True)
            v_bcast = pp.tile([128, H], BF16, tag="v_bcast")
            nc.scalar.copy(v_bcast[:], ps_vb[:])

            state = {}

            def emit_prep(s):
                nt = tiles[s]
                cov_sb = aug_lhs[s % 2]
                nc.gpsimd.dma_start(cov_sb[0:2, :nt * 128], cov_d[s])
                aug_sb = aug_rhs[s % 2]
                nc.gpsimd.dma_start(aug_sb[0:2, :], aug_d[s])
                len_sb = smp.tile([1, 1], F32, tag="len_sb")
                nc.gpsimd.dma_start(len_sb[:], lens_d[s:s + 1, :])
                ps_l = psm.tile([128, 1], F32, tag="mpsum")
                nc.tensor.matmul(ps_l[:], ones_k1[:], len_sb[:],
                                 start=True, stop=True)
                l_col = smp.tile([128, 1], F32, tag="l_col")
                nc.vector.tensor_scalar(l_col[:], ps_l[:], 1.0, None, ALU.mult)
                att_pm = bp.tile([128, 32], F32, tag="att_pm")
                state[s] = dict(cov=cov_sb, aug=aug_sb, l_col=l_col,
                                att_pm=att_pm)

            def emit_chunk(s, t0, ntile):
                st8 = state[s]
                ps = psp.tile([128, CHUNK * 512], F32, tag="feat")
                enc_ap = enc_sb[s][:].rearrange("p (k q) -> p k q", k=4)
                we_ap = we_t[:].rearrange("p (k q) -> p k q", k=4)
                for j in range(ntile):
                    t = t0 + j
                    dst = ps[:, j * 512:(j + 1) * 512]
                    if DR_EVERY and t % DR_EVERY == DR_EVERY - 1:
                        for k2 in range(2):
                            nc.tensor.matmul(
                                dst,
                                enc_ap[:, 2 * k2:2 * k2 + 2,
                                       t * 128:(t + 1) * 128],
                                we_ap[:, 2 * k2:2 * k2 + 2, :],
                                start=(k2 == 0), stop=False, perf_mode=DR)
                    else:
                        for k in range(4):
                            nc.tensor.matmul(
                                dst, enc_ap[:, k, t * 128:(t + 1) * 128],
                                we_ap[:, k, :], start=(k == 0), stop=False)
                    nc.tensor.matmul(
                        dst, st8["cov"][:, t * 128:(t + 1) * 128],
                        st8["aug"][:, :], start=False, stop=True)
                x = xp.tile([128, CHUNK * 512], BF16, tag="x")
                nc.scalar.activation(x[:, :ntile * 512], ps[:, :ntile * 512],
                                     ACTF.Tanh, scale=1.0 / SCALE)
                for j in range(ntile):
                    t = t0 + j
                    scr = scrp.tile([128, 512], BF16, tag="vscr")
                    nc.vector.scalar_tensor_tensor(
                        scr[:], x[:, j * 512:(j + 1) * 512],
                        1.0, v_bcast[:], ALU.bypass, ALU.mult,
                        accum_out=st8["att_pm"][:, t:t + 1])

            def emit_softmax_a(s):
                st8 = state[s]
                nt = tiles[s]
                att_pm, l_col = st8["att_pm"], st8["l_col"]
                pad01 = bp.tile([128, 32], F32, tag="pad01")
                nc.vector.tensor_scalar(pad01[:, :nt], iota_sb[:, :nt],
                                        l_col[:], None, ALU.is_ge)
                att_m = bp.tile([128, 32], F32, tag="att_m")
                nc.vector.scalar_tensor_tensor(
                    att_m[:, :nt], pad01[:, :nt], NEG_BIG, att_pm[:, :nt],
                    ALU.mult, ALU.add)
                exp_pm = bp.tile([128, 32], F32, tag="exp_pm")
                rowsum = smp.tile([128, 1], F32, tag="rowsum")
                nc.scalar.activation(exp_pm[:, :nt], att_m[:, :nt], ACTF.Exp,
                                     accum_out=rowsum[:])
                st8["exp_pm"] = exp_pm
                st8["rowsum"] = rowsum

            def emit_softmax_b(s):
                st8 = state.pop(s)
                nt = tiles[s]
                exp_pm, rowsum = st8["exp_pm"], st8["rowsum"]
                ps_d = psm.tile([1, 1], F32, tag="mpsum")
                nc.tensor.matmul(ps_d[:], rowsum[:], ones_col[:],
                                 start=True, stop=True)
                rinv = smp.tile([1, 1], F32, tag="rinv")
                nc.vector.reciprocal(rinv[:], ps_d[:])
                ps_r = psm.tile([128, 1], F32, tag="mpsum")
                nc.tensor.matmul(ps_r[:], ones_k1[:], rinv[:],
                                 start=True, stop=True)
                rinv_col = smp.tile([128, 1], F32, tag="rinv_col")
                nc.vector.tensor_scalar(rinv_col[:], ps_r[:], 1.0, None,
                                        ALU.mult)
                w_pm = bp.tile([128, 32], F32, tag="w_pm")
                nc.vector.tensor_scalar(w_pm[:, :nt], exp_pm[:, :nt],
                                        rinv_col[:], None, ALU.mult)
                ps_t = psm.tile([32, 128], F32, tag="mpsum")
                nc.tensor.transpose(ps_t[:nt, :], w_pm[:, :nt], ident_sb[:])
                w_sb = bp.tile([32, 128], F32, tag="w_sb")
                nc.vector.tensor_scalar(w_sb[:nt, :], ps_t[:nt, :], 1.0, None,
                                        ALU.mult)
                nc.sync.dma_start(out_d[s], w_sb[:nt, :])

            # chunk schedule: list of (slot, t0, ntile)
            sched = []
            for s, nt in enumerate(tiles):
                for t0 in range(0, nt, CHUNK):
                    sched.append((s, t0, min(CHUNK, nt - t0)))

            emit_prep(0)
            emit_prep(1)
            # softmax is split: stage a (DVE/ACT) right after the slot's
            # last chunk; stage b (PE-dependent chain) two chunks later so
            # the strict PE queue never waits on the exp.
            pend_a, pend_b = [], []
            prev_slot = 0
            for ci, (s, t0, ntile) in enumerate(sched):
                if s != prev_slot:
                    if s + 1 < N_SLOTS:
                        emit_prep(s + 1)
                    pend_a.append((ci, prev_slot))
                    prev_slot = s
                emit_chunk(s, t0, ntile)
                if pend_a and pend_a[0][0] == ci:
                    _, ps_ = pend_a.pop(0)
                    emit_softmax_a(ps_)
                    pend_b.append((ci + 2, ps_))
                if pend_b and pend_b[0][0] == ci:
                    _, ps_ = pend_b.pop(0)
                    emit_softmax_b(ps_)
            for _, ps_ in pend_b:
                emit_softmax_b(ps_)
            emit_softmax_a(N_SLOTS - 1)
            emit_softmax_b(N_SLOTS - 1)

    nc.compile()
    return nc


_NC_CACHE = {}


def _get_nc(tiles):
    key = tuple(tiles)
    if key not in _NC_CACHE:
        _NC_CACHE[key] = build_kernel(key)
    return _NC_CACHE[key]


def kernel(dec_input, enc_output, coverage_vector, text_lengths, W, b, v_w, v_b,
           _trace=False):
    dec_input = np.asarray(dec_input, np.float32)
    enc_output = np.asarray(enc_output, np.float32)
    coverage_vector = np.asarray(coverage_vector, np.float32)
    lens = np.asarray(text_lengths).astype(np.int64)
    W = np.asarray(W, np.float32)
    b = np.asarray(b, np.float32)
    v_w = np.asarray(v_w, np.float32)

    We = W[:, :H]
    Ws = W[:, H:H + D]
    Wc = W[:, H + D:]
    wc_sum = Wc.sum(axis=1)
    db = dec_input[:, 0, :] @ Ws.T + b          # [B, H] host GEMV (tiny)

    # deal batches to (core, slot) by length rank: slot s takes ranks
    # [8s, 8s+8), so the compiled per-slot cap is the max in that octet.
    order = np.argsort(-lens, kind="stable")
    assign = order.reshape(N_SLOTS, N_CORES)     # [slot, core] -> batch
    tiles = tuple(
        int(np.ceil(lens[assign[s]].max() / 128.0)) for s in range(N_SLOTS)
    )

    nc = _get_nc(tiles)

    we8 = np.ascontiguousarray(
        (We.T * SW).astype(NP_F8).reshape(4, 128, H).transpose(1, 0, 2)
        .reshape(128, 4 * H))
    iota_pm = (np.arange(32)[None, :] * 128
               + np.arange(128)[:, None]).astype(np.float32)
    ident = np.eye(128, dtype=np.float32)
    v_bf = np.ascontiguousarray(v_w[None, :].astype(NP_BF))

    in_maps = []
    for core in range(N_CORES):
        m = {"we8": we8, "v_row": v_bf, "iota_pm": iota_pm, "ident": ident}
        lens_f = np.zeros((N_SLOTS, 1), np.float32)
        for s in range(N_SLOTS):
            bidx = int(assign[s, core])
            nt = tiles[s]
            sp = nt * 128
            lens_f[s, 0] = lens[bidx]
            e8 = (enc_output[bidx, :sp, :] * SE).astype(NP_F8)
            m[f"enc8_{s}"] = np.ascontiguousarray(
                e8.reshape(sp, 4, 128).transpose(2, 1, 0))
            cov_aug = np.ones((2, sp), np.float32)
            cov_aug[1] = coverage_vector[bidx, :sp]
            m[f"cov_{s}"] = cov_aug.astype(NP_BF)
            aug = np.stack([db[bidx] * SCALE, wc_sum * SCALE])
            m[f"aug_{s}"] = aug.astype(NP_BF)
        m["lens"] = lens_f
        in_maps.append(m)

    res = run_bass_kernel_spmd(nc, in_maps, list(range(N_CORES)), trace=_trace)

    w = np.zeros((B, S), np.float32)
    for core in range(N_CORES):
        for s in range(N_SLOTS):
            bidx = int(assign[s, core])
            sp = tiles[s] * 128
            w[bidx, :sp] = res.results[core][f"out_w_{s}"].reshape(-1)
    c = coverage_vector + w
    if _trace:
        kernel.last_result = res
    return w, c


# revision 37
# speedup vs baseline: 2.4885x; 1.0881x over previous
"""Bass/Tile TRN2 kernel for nn_Attention_26388279067013 (v3).

Computes, for each batch row b:
    feat = enc @ We.T + dec @ Ws.T + cov[:,None] * Wc.sum(1) + b     [S, H]
    att  = tanh(feat) @ v_w                                          [S]
    att[s >= L_b] = -inf ; w = softmax(att) ; new_cov = cov + w

Key optimizations over the f32r baseline:
  - enc/We in fp8 e4m3 (x16 / x64 scaling) with DoubleRow matmuls:
    2 virtual-K=256 MMs per 128-row s-tile instead of 4 f32r MMs.
    The 1/1024 descale rides the tanh activation's free scale.
  - dec/bias/coverage rank-1 terms via one bf16 K=2 matmul per s-tile
    ([ones; cov] x [db; wc_sum], db = dec @ Ws.T + b computed on host).
  - masked positions (s >= L_b) have w == 0 exactly, so only
    ceil(L/128) s-tiles are computed. Batches are sorted by length and
    dealt round-robin to (core, slot) so the compiled per-slot tile
    counts (max over cores) stay small; host fills w=0 / c=cov for the
    skipped tail. new_cov = cov + w is a host-side add.
  - x = tanh(feat) in bf16 -> DVE scalar_tensor_tensor v-dot at 2x.

Sharding: 4 batch slots per core across 8 NeuronCores (SPMD).
"""

import sys

sys.path.insert(0, "/opt/trn_rl_repo")

import numpy as np
import ml_dtypes

import concourse.bacc as bacc
import concourse.tile as tile
import concourse.mybir as mybir
from concourse.bass_utils import run_bass_kernel_spmd

B, S, H, D = 32, 4096, 512, 256
N_CORES = 8
N_SLOTS = 4
F32 = mybir.dt.float32
BF16 = mybir.dt.bfloat16
F8 = mybir.dt.float8e4
ALU = mybir.AluOpType
ACTF = mybir.ActivationFunctionType
DR = mybir.MatmulPerfMode.DoubleRow
NP_F8 = ml_dtypes.float8_e4m3
NP_BF = ml_dtypes.bfloat16

SE = 16.0                     # enc fp8 scale
SW = 64.0                     # We fp8 scale
SCALE = SE * SW               # psum arrives x1024; tanh descales
NEG_BIG = -30000.0            # exp(x - 30000) == 0.0 exactly in f32
CHUNK = 3                     # s-tiles per psum tile (3 banks of 4KiB)
DR_EVERY = 3                  # of every DR_EVERY s-tiles, DR_EVERY-1 use
                              # DoubleRow and one stays plain fp8 (DR is
                              # invisible to the PE clock governor, so plain
                              # tiles must keep feeding it); 0 = all plain


def build_kernel(tiles):
    """tiles: per-slot s-tile counts (max over cores), e.g. (32, 27, 20, 10)."""
    nc = bacc.Bacc("TRN2", debug=False, num_devices=N_CORES)

    enc_d = [
        nc.dram_tensor(f"enc8_{s}", [128, 4, t * 128], F8,
                       kind="ExternalInput").ap()
        for s, t in enumerate(tiles)
    ]
    cov_d = [
        nc.dram_tensor(f"cov_{s}", [2, t * 128], BF16, kind="ExternalInput").ap()
        for s, t in enumerate(tiles)
    ]
    aug_d = [
        nc.dram_tensor(f"aug_{s}", [2, H], BF16, kind="ExternalInput").ap()
        for s in range(N_SLOTS)
    ]
    we_d = nc.dram_tensor("we8", [128, 4 * H], F8, kind="ExternalInput").ap()
    v_d = nc.dram_tensor("v_row", [1, H], BF16, kind="ExternalInput").ap()
    lens_d = nc.dram_tensor("lens", [N_SLOTS, 1], F32, kind="ExternalInput").ap()
    iota_d = nc.dram_tensor("iota_pm", [128, 32], F32, kind="ExternalInput").ap()
    ident_d = nc.dram_tensor("ident", [128, 128], F32, kind="ExternalInput").ap()
    out_d = [
        nc.dram_tensor(f"out_w_{s}", [t, 128], F32, kind="ExternalOutput").ap()
        for s, t in enumerate(tiles)
    ]

    with tile.TileContext(nc) as tc:
        with (
            tc.tile_pool(name="persist", bufs=1) as pp,
            tc.tile_pool(name="x", bufs=3) as xp,
            tc.tile_pool(name="scratch", bufs=2) as scrp,
            tc.tile_pool(name="small", bufs=4) as smp,
            tc.tile_pool(name="batch", bufs=3) as bp,
            tc.tile_pool(name="psum", bufs=2, space="PSUM") as psp,
            tc.tile_pool(name="psum_misc", bufs=2, space="PSUM") as psm,
        ):
            # ---- one-time setup ----
            we_t = pp.tile([128, 4 * H], F8, tag="we8")
            nc.scalar.dma_start(we_t[:], we_d[:, :])
            # [p, k, s] layout: plain matmuls slice one k, DoubleRow
            # matmuls slice a k-pair. Slot-0 head (2 chunks) lands first
            # so the stream starts early.
            enc_sb = []
            for s, nt in enumerate(tiles):
                enc_t = pp.tile([128, 4 * nt * 128], F8, tag=f"enc{s}")
                enc_sb.append(enc_t)
            head = min(2 * CHUNK, tiles[0]) * 128
            e0 = enc_sb[0][:].rearrange("p (k q) -> p k q", k=4)
            nc.sync.dma_start(e0[:, :, :head], enc_d[0][:, :, :head])
            if tiles[0] * 128 > head:
                nc.sync.dma_start(e0[:, :, head:], enc_d[0][:, :, head:])
            for s in (1, 2, 3):
                nc.sync.dma_start(
                    enc_sb[s][:],
                    enc_d[s].rearrange("p k q -> p (k q)"))

            ones_k1 = pp.tile([1, 128], F32, tag="ones_k1")
            nc.vector.memset(ones_k1[:], 1.0)
            ones_col = pp.tile([128, 1], F32, tag="ones_col")
            nc.vector.memset(ones_col[:], 1.0)
            ones_bf = pp.tile([1, 128], BF16, tag="ones_bf")
            nc.vector.memset(ones_bf[:], 1.0)

            # aug operands padded to K=128 (rows 2-127 zero) so the aug
            # matmul's LDWEIGHTS overlaps the preceding stream like the
            # full-K enc matmuls do (a 2-row stationary defeats the
            # weight-load pull-ahead). Double-buffered across slots.
            aug_lhs, aug_rhs = [], []
            for i in range(2):
                t = pp.tile([128, 32 * 128], BF16, tag=f"auglhs{i}")
                nc.vector.memset(t[:], 0.0)
                aug_lhs.append(t)
                t = pp.tile([128, H], BF16, tag=f"augrhs{i}")
                nc.vector.memset(t[:], 0.0)
                aug_rhs.append(t)

            vrow_sb = pp.tile([1, H], BF16, tag="vrow")
            nc.scalar.dma_start(vrow_sb[:], v_d[:, :])
            iota_sb = pp.tile([128, 32], F32, tag="iota")
            nc.scalar.dma_start(iota_sb[:], iota_d[:, :])
            ident_sb = pp.tile([128, 128], F32, tag="ident")
            nc.scalar.dma_start(ident_sb[:], ident_d[:, :])



            # v_bcast[p, o] = v_w[o]  (bf16 for the 2x DVE v-dot)
            ps_vb = psm.tile([128, H], F32, tag="mpsum")
            nc.tensor.matmul(ps_vb[:], ones_bf[:], vrow_sb[:],
                             start=True, stop=True)
            v_bcast = pp.tile([128, H], BF16, tag="v_bcast")
            nc.scalar.copy(v_bcast[:], ps_vb[:])

            state = {}

            def emit_prep(s):
                nt = tiles[s]
                cov_sb = aug_lhs[s % 2]
                nc.gpsimd.dma_start(cov_sb[0:2, :nt * 128], cov_d[s])
                aug_sb = aug_rhs[s % 2]
                nc.gpsimd.dma_start(aug_sb[0:2, :], aug_d[s])
                len_sb = smp.tile([1, 1], F32, tag="len_sb")
                nc.gpsimd.dma_start(len_sb[:], lens_d[s:s + 1, :])
                ps_l = psm.tile([128, 1], F32, tag="mpsum")
                nc.tensor.matmul(ps_l[:], ones_k1[:], len_sb[:],
                                 start=True, stop=True)
                l_col = smp.tile([128, 1], F32, tag="l_col")
                nc.vector.tensor_scalar(l_col[:], ps_l[:], 1.0, None, ALU.mult)
                att_pm = bp.tile([128, 32], F32, tag="att_pm")
                state[s] = dict(cov=cov_sb, aug=aug_sb, l_col=l_col,
                                att_pm=att_pm)

            def emit_chunk(s, t0, ntile):
                st8 = state[s]
                ps = psp.tile([128, CHUNK * 512], F32, tag="feat")
                enc_ap = enc_sb[s][:].rearrange("p (k q) -> p k q", k=4)
                we_ap = we_t[:].rearrange("p (k q) -> p k q", k=4)
                for j in range(ntile):
                    t = t0 + j
                    dst = ps[:, j * 512:(j + 1) * 512]
                    if DR_EVERY and t % DR_EVERY != 0:
                        for k2 in range(2):
                            nc.tensor.matmul(
                                dst,
                                enc_ap[:, 2 * k2:2 * k2 + 2,
                                       t * 128:(t + 1) * 128],
                                we_ap[:, 2 * k2:2 * k2 + 2, :],
                                start=(k2 == 0), stop=False, perf_mode=DR)
                    else:
                        for k in range(4):
                            nc.tensor.matmul(
                                dst, enc_ap[:, k, t * 128:(t + 1) * 128],
                                we_ap[:, k, :], start=(k == 0), stop=False)
                    nc.tensor.matmul(
                        dst, st8["cov"][:, t * 128:(t + 1) * 128],
                        st8["aug"][:, :], start=False, stop=True)
                x = xp.tile([128, CHUNK * 512], BF16, tag="x")
                nc.scalar.activation(x[:, :ntile * 512], ps[:, :ntile * 512],
                                     ACTF.Tanh, scale=1.0 / SCALE)
                for j in range(ntile):
                    t = t0 + j
                    scr = scrp.tile([128, 512], BF16, tag="vscr")
                    nc.vector.scalar_tensor_tensor(
                        scr[:], x[:, j * 512:(j + 1) * 512],
                        1.0, v_bcast[:], ALU.bypass, ALU.mult,
                        accum_out=st8["att_pm"][:, t:t + 1])

            def emit_softmax_a(s):
                st8 = state[s]
                nt = tiles[s]
                att_pm, l_col = st8["att_pm"], st8["l_col"]
                pad01 = bp.tile([128, 32], F32, tag="pad01")
                nc.vector.tensor_scalar(pad01[:, :nt], iota_sb[:, :nt],
                                        l_col[:], None, ALU.is_ge)
                att_m = bp.tile([128, 32], F32, tag="att_m")
                nc.vector.scalar_tensor_tensor(
                    att_m[:, :nt], pad01[:, :nt], NEG_BIG, att_pm[:, :nt],
                    ALU.mult, ALU.add)
                exp_pm = bp.tile([128, 32], F32, tag="exp_pm")
                rowsum = smp.tile([128, 1], F32, tag="rowsum")
                nc.scalar.activation(exp_pm[:, :nt], att_m[:, :nt], ACTF.Exp,
                                     accum_out=rowsum[:])
                st8["exp_pm"] = exp_pm
                st8["rowsum"] = rowsum

            def emit_softmax_b(s):
                st8 = state.pop(s)
                nt = tiles[s]
                exp_pm, rowsum = st8["exp_pm"], st8["rowsum"]
                ps_d = psm.tile([1, 1], F32, tag="mpsum")
                nc.tensor.matmul(ps_d[:], rowsum[:], ones_col[:],
                                 start=True, stop=True)
                rinv = smp.tile([1, 1], F32, tag="rinv")
                nc.vector.reciprocal(rinv[:], ps_d[:])
                ps_r = psm.tile([128, 1], F32, tag="mpsum")
                nc.tensor.matmul(ps_r[:], ones_k1[:], rinv[:],
                                 start=True, stop=True)
                rinv_col = smp.tile([128, 1], F32, tag="rinv_col")
                nc.vector.tensor_scalar(rinv_col[:], ps_r[:], 1.0, None,
                                        ALU.mult)
                w_pm = bp.tile([128, 32], F32, tag="w_pm")
                nc.vector.tensor_scalar(w_pm[:, :nt], exp_pm[:, :nt],
                                        rinv_col[:], None, ALU.mult)
                ps_t = psm.tile([32, 128], F32, tag="mpsum")
                nc.tensor.transpose(ps_t[:nt, :], w_pm[:, :nt], ident_sb[:])
                w_sb = bp.tile([32, 128], F32, tag="w_sb")
                nc.vector.tensor_scalar(w_sb[:nt, :], ps_t[:nt, :], 1.0, None,
                                        ALU.mult)
                nc.sync.dma_start(out_d[s], w_sb[:nt, :])

            # chunk schedule: list of (slot, t0, ntile)
            sched = []
            for s, nt in enumerate(tiles):
                for t0 in range(0, nt, CHUNK):
                    sched.append((s, t0, min(CHUNK, nt - t0)))

            emit_prep(0)
            emit_prep(1)
            # softmax is split: stage a (DVE/ACT) at the slot transition,
            # before the next slot's first chunk hits the ACT queue; stage
            # b (PE-dependent chain) one chunk later so the strict PE
            # queue never waits on the exp.
            pend_b = []
            prev_slot = 0
            for ci, (s, t0, ntile) in enumerate(sched):
                if s != prev_slot:
                    if s + 1 < N_SLOTS:
                        emit_prep(s + 1)
                    emit_softmax_a(prev_slot)
                    pend_b.append((ci + 1, prev_slot))
                    prev_slot = s
                emit_chunk(s, t0, ntile)
                if pend_b and pend_b[0][0] == ci:
                    _, ps_ = pend_b.pop(0)
                    emit_softmax_b(ps_)
            for _, ps_ in pend_b:
                emit_softmax_b(ps_)
            emit_softmax_a(N_SLOTS - 1)
            emit_softmax_b(N_SLOTS - 1)

    nc.compile()
    return nc


_NC_CACHE = {}


def _get_nc(tiles):
    key = tuple(tiles)
    if key not in _NC_CACHE:
        _NC_CACHE[key] = build_kernel(key)
    return _NC_CACHE[key]


def kernel(dec_input, enc_output, coverage_vector, text_lengths, W, b, v_w, v_b,
           _trace=False):
    dec_input = np.asarray(dec_input, np.float32)
    enc_output = np.asarray(enc_output, np.float32)
    coverage_vector = np.asarray(coverage_vector, np.float32)
    lens = np.asarray(text_lengths).astype(np.int64)
    W = np.asarray(W, np.float32)
    b = np.asarray(b, np.float32)
    v_w = np.asarray(v_w, np.float32)

    We = W[:, :H]
    Ws = W[:, H:H + D]
    Wc = W[:, H + D:]
    wc_sum = Wc.sum(axis=1)
    db = dec_input[:, 0, :] @ Ws.T + b          # [B, H] host GEMV (tiny)

    # deal batches to (core, slot) by length rank: slot s takes ranks
    # [8s, 8s+8), so the compiled per-slot cap is the max in that octet.
    order = np.argsort(-lens, kind="stable")
    assign = order.reshape(N_SLOTS, N_CORES)     # [slot, core] -> batch
    tiles = tuple(
        int(np.ceil(lens[assign[s]].max() / 128.0)) for s in range(N_SLOTS)
    )

    nc = _get_nc(tiles)

    we8 = np.ascontiguousarray(
        (We.T * SW).astype(NP_F8).reshape(4, 128, H).transpose(1, 0, 2)
        .reshape(128, 4 * H))
    iota_pm = (np.arange(32)[None, :] * 128
               + np.arange(128)[:, None]).astype(np.float32)
    ident = np.eye(128, dtype=np.float32)
    v_bf = np.ascontiguousarray(v_w[None, :].astype(NP_BF))

    in_maps = []
    for core in range(N_CORES):
        m = {"we8": we8, "v_row": v_bf, "iota_pm": iota_pm, "ident": ident}
        lens_f = np.zeros((N_SLOTS, 1), np.float32)
        for s in range(N_SLOTS):
            bidx = int(assign[s, core])
            nt = tiles[s]
            sp = nt * 128
            lens_f[s, 0] = lens[bidx]
            e8 = (enc_output[bidx, :sp, :] * SE).astype(NP_F8)
            m[f"enc8_{s}"] = np.ascontiguousarray(
                e8.reshape(sp, 4, 128).transpose(2, 1, 0))
            cov_aug = np.ones((2, sp), np.float32)
            cov_aug[1] = coverage_vector[bidx, :sp]
            m[f"cov_{s}"] = cov_aug.astype(NP_BF)
            aug = np.stack([db[bidx] * SCALE, wc_sum * SCALE])
            m[f"aug_{s}"] = aug.astype(NP_BF)
        m["lens"] = lens_f
        in_maps.append(m)

    res = run_bass_kernel_spmd(nc, in_maps, list(range(N_CORES)), trace=_trace)

    w = np.zeros((B, S), np.float32)
    for core in range(N_CORES):
        for s in range(N_SLOTS):
            bidx = int(assign[s, core])
            sp = tiles[s] * 128
            w[bidx, :sp] = res.results[core][f"out_w_{s}"].reshape(-1)
    c = coverage_vector + w
    if _trace:
        kernel.last_result = res
    return w, c
